# revision 4
# baseline (speedup 1.0000x reference)
"""CategoryAttention (softmax over heads axis) on 8 Trainium2 cores.

Sharding v2 (collective): B*L = 4096 query rows split 8 ways (512
rows/core); core c handles batch b=c//4, q-rows [(c%4)*512, ..+512).
K/V projections are SHARDED across the 4 cores of each batch (each
core projects only its own 512 k-rows) and the projected K^T / V are
exchanged with a 4-rank AllGather (replica groups [[0-3],[4-7]]).
This removes the 4x-redundant K/V projection FLOPs of the replicated
scheme (PE work 15.0 -> 8.6 GF/core).

Because the AllGather output is rank-ordered and rank r holds k-rows
[r*512, (r+1)*512), the gathered buffer is already in natural k order
-> the compiled program is identical on every core (SPMD).

Engine balance (attention sweep):
- PE: energy (row-packed head pairs) + AV matmuls
- ACT: exp only (8 activates/kt); all projection psum drains happen in
  the prologue where ACT is otherwise idle
- DVE: head-sum tree, reciprocal, rb cast, AV psum drains, and the
  normalize multiply for heads [HSPLIT:16]
- GPSIMD: normalize multiply for heads [0:HSPLIT] (otherwise idle)

psum: e_psum 2x2 banks + av_psum 2x2 banks = 8 (projections reuse
e_psum in the prologue / out-projection in the tail).
"""

import numpy as np
from contextlib import ExitStack

import concourse.bass as bass
import concourse.tile as tile
from concourse import bacc, mybir
from concourse.bass_utils import run_bass_kernel_spmd

F32 = mybir.dt.float32
BF16 = mybir.dt.bfloat16

N_CORES = 8
P = 128
D = 1024          # d_model
S = D // P        # 8 subtiles of the contraction dim
H = 16            # heads
HD = 64           # head dim
B = 2
L = 2048
LQ = L * B // N_CORES   # 512 query rows per core
LK = L                  # total key rows (per batch)
KSH = L // 4            # 512 k-rows projected per core
KTS = 128               # k tile
NKT = LK // KTS         # 16
SCALE = 1.0 / np.sqrt(HD)

import os
HSPLIT = int(os.environ.get("HSPLIT", "8"))      # heads normalized on GPSIMD
WARM0 = int(os.environ.get("WARM0", "8"))        # PE warmup groups at start
WARM1 = int(os.environ.get("WARM1", "6"))        # PE warmup during AG wait

REPLICA_GROUPS = [[0, 1, 2, 3], [4, 5, 6, 7]]


def _build(has_bias):
    nc = bacc.Bacc("TRN2", target_bir_lowering=False, debug=False, num_devices=8)

    def din(name, shape, dt):
        return nc.dram_tensor(name, shape, dt, kind="ExternalInput").ap()

    qT_d = din("qT", (P, S * LQ), BF16)
    kT_d = din("kT", (P, S * KSH), BF16)
    vT_d = din("vT", (P, S * KSH), BF16)
    wq_d = din("wq", (P, 2 * S * 512), BF16)
    wk_d = din("wk", (P, 2 * S * 512), BF16)
    wv_d = din("wv", (P, 2 * S * 512), BF16)
    wo_d = din("wo", (P, 2 * S * 512), BF16)
    bias_d = {}
    for nm in ("bq", "bk", "bv", "bo"):
        if has_bias[nm]:
            bias_d[nm] = din(nm, (1, D), F32)
    outT_d = nc.dram_tensor("outT", (P, S * LQ), F32, kind="ExternalOutput").ap()

    qT_ap = qT_d.rearrange("p (s q) -> p s q", s=S)
    kT_ap = kT_d.rearrange("p (s k) -> p s k", s=S)
    vT_ap = vT_d.rearrange("p (s k) -> p s k", s=S)
    wq_ap = wq_d.rearrange("p (h s o) -> p h s o", h=2, s=S)
    wk_ap = wk_d.rearrange("p (h s o) -> p h s o", h=2, s=S)
    wv_ap = wv_d.rearrange("p (h s o) -> p h s o", h=2, s=S)
    wo_ap = wo_d.rearrange("p (h s o) -> p h s o", h=2, s=S)
    outT_ap = outT_d.rearrange("p (j q) -> p j q", j=S)

    with tile.TileContext(nc) as tc, ExitStack() as ctx:
        # ---- persistent data tiles ----
        qt_pool = ctx.enter_context(tc.tile_pool(name="QT", bufs=1))
        kt_pool = ctx.enter_context(tc.tile_pool(name="KT", bufs=1))
        v_pool = ctx.enter_context(tc.tile_pool(name="V", bufs=1))
        QT_sb = qt_pool.tile([P, S, LQ], BF16)
        KT_sb = kt_pool.tile([P, S, LK], BF16)
        V_sb = v_pool.tile([P, NKT, 2, 512], BF16)

        any_bias = any(has_bias.values())
        bias_t = {}
        ones_t = None
        if any_bias:
            cpool = ctx.enter_context(tc.tile_pool(name="const", bufs=1))
            ones_t = cpool.tile([1, 512], F32, tag="ones")
            nc.vector.memset(ones_t[:], 1.0)
            for nm, d_ap in bias_d.items():
                t = cpool.tile([1, D], F32, tag=f"bias_{nm}")
                nc.sync.dma_start(t[:], d_ap)
                bias_t[nm] = t

        def bias_mm(ps_t, bias_name, o0, n_sz, o_on_partitions):
            if o_on_partitions:
                nc.tensor.matmul(ps_t, lhsT=bias_t[bias_name][0:1, o0:o0 + P],
                                 rhs=ones_t[0:1, :n_sz], start=False, stop=True)
            else:
                nc.tensor.matmul(ps_t, lhsT=ones_t[0:1, 0:P],
                                 rhs=bias_t[bias_name][0:1, o0:o0 + n_sz],
                                 start=False, stop=True)

        # psum pools: 8 banks total (2x2 energy/proj + 2x2 av)
        e_psum = ctx.enter_context(tc.tile_pool(name="epsum", bufs=2, space="PSUM"))
        av_psum = ctx.enter_context(tc.tile_pool(name="avpsum", bufs=2, space="PSUM"))

        # DRAM tiles for the K/V AllGather (x: 0-7 K o-tiles, 8-15 V (kt4,t))
        kv_in, kv_in_free = tc.tile([P, 16, 512], BF16, space="DRAM", name="kv_in")
        kv_out, kv_out_free = tc.tile([4, P, 16, 512], BF16, space="DRAM",
                                      name="kv_out")
        ctx.callback(kv_in_free)
        ctx.callback(kv_out_free)

        # PE warm-up during initial input DMAs (HAM clock-gate ramp).
        warm_pool = ctx.enter_context(tc.tile_pool(name="warm", bufs=1))
        wrm = warm_pool.tile([P, 512], BF16, tag="wrm")
        nc.vector.memset(wrm[:], 0.0)

        def warm_mm(n):
            for _ in range(n):
                wps = e_psum.tile([P, 2, LQ], F32, tag="e")
                for hh in range(2):
                    nc.tensor.matmul(wps[:, hh, :], lhsT=wrm[:, 0:P],
                                     rhs=wrm[:, :], start=True, stop=True)

        warm_mm(WARM0)

        # ---------------- prologue: sharded projections ----------------
        # K shard -> KT_sb[:, :, 0:512] -> kv_in K part
        with tc.tile_pool(name="kstream", bufs=1) as kspool, \
             tc.tile_pool(name="kwpool", bufs=2) as kwpool:
            kin = kspool.tile([P, S, KSH], BF16, tag="kin")
            nc.sync.dma_start(kin[:], kT_ap)
            wk_h = []
            for wh in range(2):
                t = kwpool.tile([P, S, 512], BF16, tag="wk")
                nc.sync.dma_start(t[:], wk_ap[:, wh])
                wk_h.append(t)
            for jp in range(4):
                ps = e_psum.tile([P, 2, 512], F32, tag="e")
                for hh in range(2):
                    j = 2 * jp + hh
                    w_t = wk_h[j // 4]
                    jl = j % 4
                    for s in range(S):
                        nc.tensor.matmul(
                            ps[:, hh, :],
                            lhsT=w_t[:, s, jl * P:(jl + 1) * P],
                            rhs=kin[:, s, :],
                            start=(s == 0),
                            stop=(s == S - 1 and not has_bias["bk"]),
                        )
                    if has_bias["bk"]:
                        bias_mm(ps[:, hh, :], "bk", j * P, KSH, True)
                nc.scalar.copy(KT_sb[:, 2 * jp:2 * jp + 2, 0:KSH], ps[:])
        nc.sync.dma_start(kv_in[:, 0:8, :], KT_sb[:, :, 0:KSH])

        # V shard -> V_sb[:, 0:4] -> kv_in V part
        with tc.tile_pool(name="vstream", bufs=1) as vspool, \
             tc.tile_pool(name="vwpool", bufs=2) as vwpool:
            vin = vspool.tile([P, S, KSH], BF16, tag="vin")
            nc.sync.dma_start(vin[:], vT_ap)
            wv_h = []
            for wh in range(2):
                t = vwpool.tile([P, S, 512], BF16, tag="wv")
                nc.sync.dma_start(t[:], wv_ap[:, wh])
                wv_h.append(t)
            for kt4 in range(4):
                ps = e_psum.tile([P, 2, 512], F32, tag="e")
                for t in range(2):
                    for s in range(S):
                        nc.tensor.matmul(
                            ps[:, t, :],
                            lhsT=vin[:, s, kt4 * P:(kt4 + 1) * P],
                            rhs=wv_h[t][:, s, :],
                            start=(s == 0),
                            stop=(s == S - 1 and not has_bias["bv"]),
                        )
                    if has_bias["bv"]:
                        bias_mm(ps[:, t, :], "bv", t * 512, 512, False)
                nc.scalar.copy(V_sb[:, kt4, :, :], ps[:])
        nc.sync.dma_start(kv_in[:, 8:16, :], V_sb[:, 0:4, :, :].rearrange(
            "p n t y -> p (n t) y"))

        # AllGather projected K/V within each batch's 4-core group.
        nc.gpsimd.collective_compute(
            "AllGather",
            mybir.AluOpType.bypass,
            replica_groups=REPLICA_GROUPS,
            ins=[kv_in[:]],
            outs=[kv_out[:]],
        )

        # Q projection (overlaps the AllGather)
        with tc.tile_pool(name="qstream", bufs=1) as qspool, \
             tc.tile_pool(name="qwpool", bufs=2) as qwpool:
            qin = qspool.tile([P, S, LQ], BF16, tag="qin")
            nc.sync.dma_start(qin[:], qT_ap)
            wq_h = []
            for wh in range(2):
                t = qwpool.tile([P, S, 512], BF16, tag="wq")
                nc.sync.dma_start(t[:], wq_ap[:, wh])
                wq_h.append(t)
            for jp in range(4):
                ps = e_psum.tile([P, 2, LQ], F32, tag="e")
                for hh in range(2):
                    j = 2 * jp + hh
                    w_t = wq_h[j // 4]
                    jl = j % 4
                    for s in range(S):
                        nc.tensor.matmul(
                            ps[:, hh, :],
                            lhsT=w_t[:, s, jl * P:(jl + 1) * P],
                            rhs=qin[:, s, :],
                            start=(s == 0),
                            stop=(s == S - 1 and not has_bias["bq"]),
                        )
                    if has_bias["bq"]:
                        bias_mm(ps[:, hh, :], "bq", j * P, LQ, True)
                nc.scalar.copy(QT_sb[:, 2 * jp:2 * jp + 2, :], ps[:])

        # warm-keeper while the AllGather completes
        warm_mm(WARM1)

        # Readback: gathered K first (energy needs it first), then V.
        for c in range(4):
            nc.sync.dma_start(KT_sb[:, :, c * KSH:(c + 1) * KSH],
                              kv_out[c, :, 0:8, :])
        for c in range(4):
            nc.sync.dma_start(V_sb[:, c * 4:(c + 1) * 4, :, :],
                              kv_out[c, :, 8:16, :].rearrange(
                                  "p (n t) y -> p n t y", t=2))

        # ---- attention-era pools ----
        wo_pool = ctx.enter_context(tc.tile_pool(name="wo", bufs=1))
        attn_pool = ctx.enter_context(tc.tile_pool(name="attn", bufs=4))
        tree_pool = ctx.enter_context(tc.tile_pool(name="tree", bufs=2))
        den_pool = ctx.enter_context(tc.tile_pool(name="den", bufs=2))
        r_pool = ctx.enter_context(tc.tile_pool(name="r", bufs=2))
        rb_pool = ctx.enter_context(tc.tile_pool(name="rb", bufs=2))
        ctx_pool = ctx.enter_context(tc.tile_pool(name="ctx", bufs=1))
        osb_pool = ctx.enter_context(tc.tile_pool(name="osb", bufs=2))

        ctx_sb = ctx_pool.tile([P, S, LQ], BF16)

        # ---------------- attention ----------------
        def softmax_kt(kt):
            """Energy (16 heads, row-packed pairs) -> exp -> normalized attn."""
            attn_t = attn_pool.tile([P, H, LQ], BF16, tag="attn")
            for g in range(8):
                eps = e_psum.tile([P, 2, LQ], F32, tag="e")
                for hh in range(2):
                    p0 = HD * hh
                    nc.tensor.matmul(
                        eps[:, hh, :],
                        lhsT=KT_sb[p0:p0 + HD, g, kt * KTS:(kt + 1) * KTS],
                        rhs=QT_sb[p0:p0 + HD, g, :],
                        start=True,
                        stop=True,
                    )
                nc.scalar.activation(attn_t[:, g * 2:(g + 1) * 2, :], eps[:],
                                     mybir.ActivationFunctionType.Exp,
                                     scale=float(SCALE))
            # den = sum over heads (bf16 tree at DVE 2x; final add f32)
            t1 = tree_pool.tile([P, 4, LQ], BF16, tag="t1")
            with nc.allow_low_precision(reason="bf16 head-sum tree"):
                nc.vector.tensor_add(t1[:], attn_t[:, 0:4, :], attn_t[:, 4:8, :])
                nc.vector.tensor_add(t1[:], t1[:], attn_t[:, 8:12, :])
                nc.vector.tensor_add(t1[:], t1[:], attn_t[:, 12:16, :])
                nc.vector.tensor_add(t1[:, 0:2, :], t1[:, 0:2, :], t1[:, 2:4, :])
            den = den_pool.tile([P, LQ], F32, tag="den")
            nc.vector.tensor_add(den[:], t1[:, 0, :], t1[:, 1, :])
            r32 = r_pool.tile([P, LQ], F32, tag="r")
            nc.vector.reciprocal_approx_fast(r32[:], den[:])
            rb = rb_pool.tile([P, LQ], BF16, tag="rb")
            with nc.allow_low_precision(reason="bf16 reciprocal"):
                nc.vector.tensor_copy(rb[:], r32[:])
            if HSPLIT > 0:
                nc.gpsimd.tensor_mul(
                    attn_t[:, 0:HSPLIT, :], attn_t[:, 0:HSPLIT, :],
                    rb[:, None, :].to_broadcast((P, HSPLIT, LQ)))
            if HSPLIT < H:
                nc.vector.tensor_mul(
                    attn_t[:, HSPLIT:H, :], attn_t[:, HSPLIT:H, :],
                    rb[:, None, :].to_broadcast((P, H - HSPLIT, LQ)))
            return attn_t

        def av_group(u, c0, attn_list, first):
            """One av psum tile: heads 4u..4u+3, full q, over 2 k-tiles."""
            avp = av_psum.tile([P, 2, LQ], F32, tag="av")
            for ci in range(2):
                kt = c0 + ci
                for hh in range(4):
                    h = 4 * u + hh
                    i, p0 = hh // 2, HD * (hh % 2)
                    nc.tensor.matmul(
                        avp[p0:p0 + HD, i, :],
                        lhsT=V_sb[:, kt, h // 8, (h % 8) * HD:(h % 8 + 1) * HD],
                        rhs=attn_list[ci][:, h, :],
                        start=(ci == 0),
                        stop=(ci == 1),
                    )
            with nc.allow_low_precision(reason="bf16 ctx accumulate"):
                if first:
                    nc.vector.tensor_copy(ctx_sb[:, 2 * u:2 * u + 2, :],
                                          avp[:, :, :])
                else:
                    nc.vector.tensor_add(ctx_sb[:, 2 * u:2 * u + 2, :],
                                         ctx_sb[:, 2 * u:2 * u + 2, :],
                                         avp[:, :, :])

        wo_tiles = []

        def dma_wo0():
            t = wo_pool.tile([P, S, 512], BF16, tag="wo")
            nc.sync.dma_start(t[:], wo_ap[:, 0])
            wo_tiles.append(t)

        # sweep: av for pair (kt-2)//2 is emitted at kt (lag ~= 1 pair)
        attn_tiles = {}
        for kt in range(NKT):
            attn_tiles[kt] = softmax_kt(kt)
            if kt >= 2:
                pair = (kt - 2) // 2
                u0 = 0 if kt % 2 == 0 else 2
                alist = [attn_tiles[2 * pair], attn_tiles[2 * pair + 1]]
                av_group(u0, 2 * pair, alist, pair == 0)
                av_group(u0 + 1, 2 * pair, alist, pair == 0)
            if kt == 10:
                dma_wo0()
        # final pair
        alist = [attn_tiles[NKT - 2], attn_tiles[NKT - 1]]
        for u in range(4):
            av_group(u, NKT - 2, alist, False)

        # ---------------- output projection ----------------
        for j4 in range(2):
            if j4 == 0 and wo_tiles:
                woh = wo_tiles[0]
            else:
                woh = wo_pool.tile([P, S, 512], BF16, tag="wo")
                nc.sync.dma_start(woh[:], wo_ap[:, j4])
            for j2 in range(2):
                po = e_psum.tile([P, 2, LQ], F32, tag="e")
                for jj in range(2):
                    j = j4 * 4 + j2 * 2 + jj
                    jl = j2 * 2 + jj
                    for s in range(S):
                        nc.tensor.matmul(
                            po[:, jj, :],
                            lhsT=woh[:, s, jl * P:(jl + 1) * P],
                            rhs=ctx_sb[:, s, :],
                            start=(s == 0),
                            stop=(s == S - 1 and not has_bias["bo"]),
                        )
                    if has_bias["bo"]:
                        bias_mm(po[:, jj, :], "bo", j * P, LQ, True)
                osb = osb_pool.tile([P, 2, LQ], F32, tag="osb")
                nc.scalar.copy(osb[:], po[:])
                j0 = j4 * 4 + j2 * 2
                nc.sync.dma_start(outT_ap[:, j0:j0 + 2, :], osb[:])

    nc.compile()
    return nc


_cache = {}


def _get_program(has_bias):
    key = (HSPLIT, tuple(sorted(has_bias.items())))
    if key not in _cache:
        _cache[key] = _build(has_bias)
    return _cache[key]


def _part_major(x):
    n = x.shape[1]
    return np.ascontiguousarray(
        x.reshape(S, P, n).transpose(1, 0, 2).reshape(P, S * n))


def _chunked(x, width=512):
    """[D, N] -> [P, N//width, S, width] per-chunk contiguous layout."""
    n = x.shape[1]
    nch = n // width
    y = x.reshape(S, P, nch, width).transpose(1, 2, 0, 3)
    return np.ascontiguousarray(y.reshape(P, nch * S * width))


def _bf16(x):
    import ml_dtypes
    return np.ascontiguousarray(x).astype(ml_dtypes.bfloat16)


def prepare_inputs(query, key, value, Wq_w, Wq_b, Wk_w, Wk_b, Wv_w, Wv_b,
                   Wo_w, Wo_b):
    query = np.asarray(query, dtype=np.float32)
    key = np.asarray(key, dtype=np.float32)
    value = np.asarray(value, dtype=np.float32)
    w = {
        "wq": _bf16(_chunked(np.ascontiguousarray(np.asarray(Wq_w, np.float32).T))),
        "wk": _bf16(_chunked(np.ascontiguousarray(np.asarray(Wk_w, np.float32).T))),
        "wv": _bf16(_chunked(np.ascontiguousarray(np.asarray(Wv_w, np.float32).T))),
        "wo": _bf16(_chunked(np.ascontiguousarray(np.asarray(Wo_w, np.float32).T))),
    }
    biases = {"bq": np.asarray(Wq_b, np.float32), "bk": np.asarray(Wk_b, np.float32),
              "bv": np.asarray(Wv_b, np.float32), "bo": np.asarray(Wo_b, np.float32)}
    has_bias = {nm: bool(np.any(b)) for nm, b in biases.items()}

    in_maps = []
    for c in range(N_CORES):
        b, qc = c // (N_CORES // B), c % (N_CORES // B)
        sl = slice(qc * LQ, (qc + 1) * LQ)
        m = {
            "qT": _bf16(_part_major(np.ascontiguousarray(query[b, sl, :].T))),
            "kT": _bf16(_part_major(np.ascontiguousarray(key[b, sl, :].T))),
            "vT": _bf16(_part_major(np.ascontiguousarray(value[b, sl, :].T))),
            **w,
        }
        for nm, hb in has_bias.items():
            if hb:
                m[nm] = biases[nm].reshape(1, D)
        in_maps.append(m)
    return in_maps, has_bias


def gather_output(results):
    out = np.empty((B, L, D), dtype=np.float32)
    for c in range(N_CORES):
        b, qc = c // (N_CORES // B), c % (N_CORES // B)
        oT = results[c]["outT"].reshape(P, S, LQ).transpose(1, 0, 2).reshape(D, LQ)
        out[b, qc * LQ:(qc + 1) * LQ, :] = oT.T
    return out


def kernel(**inputs) -> np.ndarray:
    in_maps, has_bias = prepare_inputs(**inputs)
    nc = _get_program(has_bias)
    res = run_bass_kernel_spmd(nc, in_maps, list(range(N_CORES)))
    return gather_output(res.results)


# revision 9
# speedup vs baseline: 1.0564x; 1.0564x over previous
"""CategoryAttention (softmax over heads axis) on 8 Trainium2 cores.

Sharding v3 (k-shard + ReduceScatter): core c handles batch b=c//4 and
K-SHARD r=c%4 (k-rows [r*512, (r+1)*512)). Each core projects only its
own K/V shard (no redundancy), projects the FULL batch Q (4x redundant,
but 41us cheaper than gathering K/V), computes energy/exp/softmax-over-
heads for (all 2048 q) x (own 512 k) -- the head-axis softmax is local
per (k,q), so k-sharding needs no softmax comm -- and accumulates a
PARTIAL AV context. Partials are summed across the 4 cores of a batch
with a per-q-block ReduceScatter (4 pipelined 1MB collectives, [[0-3],
[4-7]] replica groups); rank r receives q-strip r of each 512-row
q-block, runs the output projection on it, and the host reassembles the
strips. The RS wire cost (~40us/call) fully overlaps the next q-block's
compute; only the last RS sits on the tail.

Engine balance per (q-block, k-tile) unit (16 units total):
- PE: energy (row-packed head pairs), AV (4-kt accumulated, 2-head
  row-packed psum tiles), Q-projection of the next q-block as filler
- ACT (the ~9.2us/unit wall): exp only -- 8 activates of (1024+352)/1.2
  ns; projection drains live in the prologue / out-proj drains per strip
- DVE: head-sum tree, reciprocal, normalize for ODD k-tiles (full
  16-head multiply stays on the known-fast broadcast path), AV drains
- GPSIMD: normalize for EVEN k-tiles + den add + rb cast (kt-parity
  split avoids same-tile GPS/DVE contention)

psum banks: energy 2x2 + av 2x1 + out-proj 2x1 = 8.
"""

import numpy as np
from contextlib import ExitStack

import concourse.bass as bass
import concourse.tile as tile
from concourse import bacc, mybir
from concourse.bass_utils import run_bass_kernel_spmd

F32 = mybir.dt.float32
BF16 = mybir.dt.bfloat16

N_CORES = 8
P = 128
D = 1024          # d_model
S = D // P        # 8 subtiles of the contraction dim
H = 16            # heads
HD = 64           # head dim
B = 2
L = 2048
NQB = 4                 # q blocks per batch
QB = L // NQB           # 512 q rows per block
QS = QB // 4            # 128-row q strip returned per core per block
KSH = L // 4            # 512 k-rows per core
KTS = 128               # k tile
NKT = KSH // KTS        # 4 own k-tiles
SCALE = 1.0 / np.sqrt(HD)

import os
NORM_GPS_PARITY = int(os.environ.get("NORM_GPS_PARITY", "1"))  # even kt on GPS
GPS_SMALL = int(os.environ.get("GPS_SMALL", "1"))  # den add + rb cast on GPS
WARM0 = int(os.environ.get("WARM0", "8"))

REPLICA_GROUPS = [[0, 1, 2, 3], [4, 5, 6, 7]]


def _build(has_bias):
    nc = bacc.Bacc("TRN2", target_bir_lowering=False, debug=False, num_devices=8)

    def din(name, shape, dt):
        return nc.dram_tensor(name, shape, dt, kind="ExternalInput").ap()

    qT_d = din("qT", (P, NQB * S * QB), BF16)   # full-batch Q^T, chunked
    kT_d = din("kT", (P, S * KSH), BF16)        # input K^T shard
    vT_d = din("vT", (P, S * KSH), BF16)
    wq_d = din("wq", (P, 2 * S * 512), BF16)
    wk_d = din("wk", (P, 2 * S * 512), BF16)
    wv_d = din("wv", (P, 2 * S * 512), BF16)
    wo_d = din("wo", (P, 2 * S * 512), BF16)
    bias_d = {}
    for nm in ("bq", "bk", "bv", "bo"):
        if has_bias[nm]:
            bias_d[nm] = din(nm, (1, D), F32)
    # out: 4 q-strips of 128 rows, [p, j, (qb, 128)]
    outT_d = nc.dram_tensor("outT", (P, S * NQB * QS), F32, kind="ExternalOutput").ap()

    qT_ap = qT_d.rearrange("p (c s q) -> p c s q", c=NQB, s=S)
    kT_ap = kT_d.rearrange("p (s k) -> p s k", s=S)
    vT_ap = vT_d.rearrange("p (s k) -> p s k", s=S)
    wq_ap = wq_d.rearrange("p (h s o) -> p h s o", h=2, s=S)
    wk_ap = wk_d.rearrange("p (h s o) -> p h s o", h=2, s=S)
    wv_ap = wv_d.rearrange("p (h s o) -> p h s o", h=2, s=S)
    wo_ap = wo_d.rearrange("p (h s o) -> p h s o", h=2, s=S)
    outT_ap = outT_d.rearrange("p (j c q) -> p j c q", j=S, c=NQB)

    with tile.TileContext(nc) as tc, ExitStack() as ctx:
        # ---- persistent data tiles ----
        qt_pool = ctx.enter_context(tc.tile_pool(name="QT", bufs=1))
        kt_pool = ctx.enter_context(tc.tile_pool(name="KT", bufs=1))
        v_pool = ctx.enter_context(tc.tile_pool(name="V", bufs=1))
        QT_sb = qt_pool.tile([P, NQB, S, QB], BF16)
        KT_sb = kt_pool.tile([P, S, KSH], BF16)
        V_sb = v_pool.tile([P, NKT, 2, 512], BF16)

        any_bias = any(has_bias.values())
        bias_t = {}
        ones_t = None
        if any_bias:
            cpool = ctx.enter_context(tc.tile_pool(name="const", bufs=1))
            ones_t = cpool.tile([1, 512], F32, tag="ones")
            nc.vector.memset(ones_t[:], 1.0)
            for nm, d_ap in bias_d.items():
                t = cpool.tile([1, D], F32, tag=f"bias_{nm}")
                nc.sync.dma_start(t[:], d_ap)
                bias_t[nm] = t

        def bias_mm(ps_t, bias_name, o0, n_sz, o_on_partitions):
            if o_on_partitions:
                nc.tensor.matmul(ps_t, lhsT=bias_t[bias_name][0:1, o0:o0 + P],
                                 rhs=ones_t[0:1, :n_sz], start=False, stop=True)
            else:
                nc.tensor.matmul(ps_t, lhsT=ones_t[0:1, 0:P],
                                 rhs=bias_t[bias_name][0:1, o0:o0 + n_sz],
                                 start=False, stop=True)

        # psum: energy 2x2 banks + av 2x1 + out-proj 2x1 = 8
        e_psum = ctx.enter_context(tc.tile_pool(name="epsum", bufs=2, space="PSUM"))
        av_psum = ctx.enter_context(tc.tile_pool(name="avpsum", bufs=2, space="PSUM"))
        o_psum = ctx.enter_context(tc.tile_pool(name="opsum", bufs=2, space="PSUM"))

        # DRAM tiles for the per-q-block ReduceScatter.
        rs_in = []
        rs_out = []
        for qb in range(NQB):
            ti, fi = tc.tile([4, P, S, QS], BF16, space="DRAM", name=f"rsin{qb}")
            to, fo = tc.tile([P, S, QS], BF16, space="DRAM", name=f"rsout{qb}")
            ctx.callback(fi)
            ctx.callback(fo)
            rs_in.append(ti)
            rs_out.append(to)

        # PE warm-up during initial input DMAs (HAM clock-gate ramp).
        warm_pool = ctx.enter_context(tc.tile_pool(name="warm", bufs=1))
        wrm = warm_pool.tile([P, 128], BF16, tag="wrm")
        nc.vector.memset(wrm[:], 0.0)
        for _ in range(WARM0):
            wps = e_psum.tile([P, 2, QB], F32, tag="e")
            for hh in range(2):
                nc.tensor.matmul(wps[:, hh, 0:128], lhsT=wrm[:, 0:P],
                                 rhs=wrm[:, :], start=True, stop=True)

        # ---------------- prologue: K/V shard projections ----------------
        with tc.tile_pool(name="kstream", bufs=1) as kspool, \
             tc.tile_pool(name="kwpool", bufs=2) as kwpool:
            kin = kspool.tile([P, S, KSH], BF16, tag="kin")
            nc.sync.dma_start(kin[:], kT_ap)
            wk_h = []
            for wh in range(2):
                t = kwpool.tile([P, S, 512], BF16, tag="wk")
                nc.sync.dma_start(t[:], wk_ap[:, wh])
                wk_h.append(t)
            for jp in range(4):
                ps = e_psum.tile([P, 2, KSH], F32, tag="e")
                for hh in range(2):
                    j = 2 * jp + hh
                    w_t = wk_h[j // 4]
                    jl = j % 4
                    for s in range(S):
                        nc.tensor.matmul(
                            ps[:, hh, :],
                            lhsT=w_t[:, s, jl * P:(jl + 1) * P],
                            rhs=kin[:, s, :],
                            start=(s == 0),
                            stop=(s == S - 1 and not has_bias["bk"]),
                        )
                    if has_bias["bk"]:
                        bias_mm(ps[:, hh, :], "bk", j * P, KSH, True)
                nc.scalar.copy(KT_sb[:, 2 * jp:2 * jp + 2, :], ps[:])

        with tc.tile_pool(name="vstream", bufs=1) as vspool, \
             tc.tile_pool(name="vwpool", bufs=2) as vwpool:
            vin = vspool.tile([P, S, KSH], BF16, tag="vin")
            nc.sync.dma_start(vin[:], vT_ap)
            wv_h = []
            for wh in range(2):
                t = vwpool.tile([P, S, 512], BF16, tag="wv")
                nc.sync.dma_start(t[:], wv_ap[:, wh])
                wv_h.append(t)
            for kt4 in range(NKT):
                ps = e_psum.tile([P, 2, 512], F32, tag="e")
                for t in range(2):
                    for s in range(S):
                        nc.tensor.matmul(
                            ps[:, t, :],
                            lhsT=vin[:, s, kt4 * P:(kt4 + 1) * P],
                            rhs=wv_h[t][:, s, :],
                            start=(s == 0),
                            stop=(s == S - 1 and not has_bias["bv"]),
                        )
                    if has_bias["bv"]:
                        bias_mm(ps[:, t, :], "bv", t * 512, 512, False)
                nc.scalar.copy(V_sb[:, kt4, :, :], ps[:])

        # ---- attention-era pools ----
        pw_pool = ctx.enter_context(tc.tile_pool(name="pw", bufs=1))
        qin_pool = ctx.enter_context(tc.tile_pool(name="qin", bufs=1))
        attn_pool = ctx.enter_context(tc.tile_pool(name="attn", bufs=5))
        tree_pool = ctx.enter_context(tc.tile_pool(name="tree", bufs=2))
        den_pool = ctx.enter_context(tc.tile_pool(name="den", bufs=2))
        r_pool = ctx.enter_context(tc.tile_pool(name="r", bufs=2))
        rb_pool = ctx.enter_context(tc.tile_pool(name="rb", bufs=2))
        ctx_pool = ctx.enter_context(tc.tile_pool(name="ctx", bufs=1))
        ob_pool = ctx.enter_context(tc.tile_pool(name="ob", bufs=2))
        osb_pool = ctx.enter_context(tc.tile_pool(name="osb", bufs=2))

        wq_h = []
        for wh in range(2):
            t = pw_pool.tile([P, S, 512], BF16, tag=f"pw{wh}")
            nc.sync.dma_start(t[:], wq_ap[:, wh])
            wq_h.append(t)

        # ---- Q projection of one q-block (8 o-tiles; emitted piecewise) ----
        qctx = {}

        def q_proj_start(qb):
            qin = qin_pool.tile([P, S, QB], BF16, tag="qin")
            nc.sync.dma_start(qin[:], qT_ap[:, qb])
            qctx[qb] = qin

        def q_proj_pair(qb, jp):
            qin = qctx[qb]
            ps = e_psum.tile([P, 2, QB], F32, tag="e")
            for hh in range(2):
                j = 2 * jp + hh
                w_t = wq_h[j // 4]
                jl = j % 4
                for s in range(S):
                    nc.tensor.matmul(
                        ps[:, hh, :],
                        lhsT=w_t[:, s, jl * P:(jl + 1) * P],
                        rhs=qin[:, s, :],
                        start=(s == 0),
                        stop=(s == S - 1 and not has_bias["bq"]),
                    )
                if has_bias["bq"]:
                    bias_mm(ps[:, hh, :], "bq", j * P, QB, True)
            nc.scalar.copy(QT_sb[:, qb, 2 * jp:2 * jp + 2, :], ps[:])

        # ---------------- attention unit ----------------
        def softmax_unit(qb, kt, fillers=()):
            """energy -> exp -> head-sum -> reciprocal -> normalize.
            fillers: PE work thunks interleaved after exp groups."""
            attn_t = attn_pool.tile([P, H, QB], BF16, tag="attn")
            fi = 0
            for g in range(8):
                eps = e_psum.tile([P, 2, QB], F32, tag="e")
                for hh in range(2):
                    p0 = HD * hh
                    nc.tensor.matmul(
                        eps[:, hh, :],
                        lhsT=KT_sb[p0:p0 + HD, g, kt * KTS:(kt + 1) * KTS],
                        rhs=QT_sb[p0:p0 + HD, qb, g, :],
                        start=True,
                        stop=True,
                    )
                nc.scalar.activation(attn_t[:, g * 2:(g + 1) * 2, :], eps[:],
                                     mybir.ActivationFunctionType.Exp,
                                     scale=float(SCALE))
                while fi * 8 < (g + 1) * len(fillers):
                    fillers[fi]()
                    fi += 1
            t1 = tree_pool.tile([P, 4, QB], BF16, tag="t1")
            with nc.allow_low_precision(reason="bf16 head-sum tree"):
                nc.vector.tensor_add(t1[:], attn_t[:, 0:4, :], attn_t[:, 4:8, :])
                nc.vector.tensor_add(t1[:], t1[:], attn_t[:, 8:12, :])
                nc.vector.tensor_add(t1[:], t1[:], attn_t[:, 12:16, :])
                nc.vector.tensor_add(t1[:, 0:2, :], t1[:, 0:2, :], t1[:, 2:4, :])
            den = den_pool.tile([P, QB], F32, tag="den")
            if GPS_SMALL:
                nc.gpsimd.tensor_add(den[:], t1[:, 0, :], t1[:, 1, :])
            else:
                nc.vector.tensor_add(den[:], t1[:, 0, :], t1[:, 1, :])
            r32 = r_pool.tile([P, QB], F32, tag="r")
            nc.vector.reciprocal_approx_fast(r32[:], den[:])
            rb = rb_pool.tile([P, QB], BF16, tag="rb")
            with nc.allow_low_precision(reason="bf16 reciprocal"):
                if GPS_SMALL:
                    nc.gpsimd.tensor_copy(rb[:], r32[:])
                else:
                    nc.vector.tensor_copy(rb[:], r32[:])
            eng = nc.gpsimd if (NORM_GPS_PARITY and kt % 2 == 0) else nc.vector
            eng.tensor_mul(attn_t[:], attn_t[:],
                           rb[:, None, :].to_broadcast((P, H, QB)))
            return attn_t

        ctx_tiles = {}

        def av_pair(qb, u, attn_list):
            """Heads 2u,2u+1 x all 4 own k-tiles -> one [128,512] psum tile
            (head parity on partition halves) -> ctx j-subtile u."""
            if u == 0:
                ctx_tiles[qb] = ctx_pool.tile([P, S, QB], BF16, tag="ctx", name="ctxp")
            avp = av_psum.tile([P, QB], F32, tag="av")
            for kt in range(NKT):
                for hh in range(2):
                    h = 2 * u + hh
                    p0 = HD * hh
                    nc.tensor.matmul(
                        avp[p0:p0 + HD, :],
                        lhsT=V_sb[:, kt, h // 8, (h % 8) * HD:(h % 8 + 1) * HD],
                        rhs=attn_list[kt][:, h, :],
                        start=(kt == 0),
                        stop=(kt == NKT - 1),
                    )
            with nc.allow_low_precision(reason="bf16 ctx partial"):
                nc.vector.tensor_copy(ctx_tiles[qb][:, u, :], avp[:])
            # stream this j-subtile to the RS input buffer immediately
            for c in range(4):
                nc.sync.dma_start(rs_in[qb][c, :, u, :],
                                  ctx_tiles[qb][:, u, c * QS:(c + 1) * QS])

        def rs_kick(qb):
            nc.gpsimd.collective_compute(
                "ReduceScatter",
                mybir.AluOpType.add,
                replica_groups=REPLICA_GROUPS,
                ins=[rs_in[qb][:]],
                outs=[rs_out[qb][:]],
            )

        def out_proj(qb):
            """Output projection of this core's 128-row strip of q-block qb."""
            ob = ob_pool.tile([P, S, QS], BF16, tag="ob")
            nc.sync.dma_start(ob[:], rs_out[qb][:])
            for j4 in range(2):
                woh = wo_tiles[j4]
                for j2 in range(2):
                    po = o_psum.tile([P, 2, QS], F32, tag="o")
                    for jj in range(2):
                        j = j4 * 4 + j2 * 2 + jj
                        jl = j2 * 2 + jj
                        for s in range(S):
                            nc.tensor.matmul(
                                po[:, jj, :],
                                lhsT=woh[:, s, jl * P:(jl + 1) * P],
                                rhs=ob[:, s, :],
                                start=(s == 0),
                                stop=(s == S - 1 and not has_bias["bo"]),
                            )
                        if has_bias["bo"]:
                            bias_mm(po[:, jj, :], "bo", j * P, QS, True)
                    osb = osb_pool.tile([P, 2, QS], F32, tag="osb")
                    nc.scalar.copy(osb[:], po[:])
                    j0 = j4 * 4 + j2 * 2
                    nc.sync.dma_start(outT_ap[:, j0:j0 + 2, qb, :], osb[:])

        wo_tiles = []

        def dma_wo():
            for wh in range(2):
                t = pw_pool.tile([P, S, 512], BF16, tag=f"pw{wh}")
                nc.sync.dma_start(t[:], wo_ap[:, wh])
                wo_tiles.append(t)

        # ---------------- sweep ----------------
        # Unit (qb, kt). Fillers per unit:
        #   kt0: the 8 av_pairs of qb-1 (interleaved between exp groups),
        #        then rs_kick(qb-1); kt1: qin DMA of qb+1; kt2/kt3: Q-proj
        #        o-pairs of qb+1; qb3-kt0: wo DMA (pw pool free after the
        #        last Q-proj pair); qb3-kt1/kt3 + tail: out-proj strips.
        q_proj_start(0)
        for jp in range(4):
            q_proj_pair(0, jp)

        attn_tiles = {}
        for qb in range(NQB):
            attn_tiles[qb] = []
            for kt in range(NKT):
                fillers = []
                if kt == 0 and qb > 0:
                    prev = attn_tiles[qb - 1]
                    fillers = [
                        (lambda u=u, p=prev, q=qb - 1: av_pair(q, u, p))
                        for u in range(8)
                    ]
                attn_tiles[qb].append(softmax_unit(qb, kt, fillers))
                if kt == 0 and qb > 0:
                    rs_kick(qb - 1)
                if kt == 0 and qb == NQB - 1:
                    dma_wo()
                if kt == 1 and qb + 1 < NQB:
                    q_proj_start(qb + 1)
                if kt in (2, 3) and qb + 1 < NQB:
                    q_proj_pair(qb + 1, 2 * (kt - 2))
                    q_proj_pair(qb + 1, 2 * (kt - 2) + 1)
                if qb == NQB - 1 and kt == 1:
                    out_proj(0)
                if qb == NQB - 1 and kt == 3:
                    out_proj(1)

        # tail: av + RS + out-proj of the last block(s)
        prev = attn_tiles[NQB - 1]
        for u in range(8):
            av_pair(NQB - 1, u, prev)
        rs_kick(NQB - 1)
        out_proj(NQB - 2)
        out_proj(NQB - 1)

    nc.compile()
    return nc


_cache = {}


def _get_program(has_bias):
    key = (NORM_GPS_PARITY, GPS_SMALL, tuple(sorted(has_bias.items())))
    if key not in _cache:
        _cache[key] = _build(has_bias)
    return _cache[key]


def _part_major(x):
    n = x.shape[1]
    return np.ascontiguousarray(
        x.reshape(S, P, n).transpose(1, 0, 2).reshape(P, S * n))


def _chunked(x, width=512):
    """[D, N] -> [P, N//width, S, width] per-chunk contiguous layout."""
    n = x.shape[1]
    nch = n // width
    y = x.reshape(S, P, nch, width).transpose(1, 2, 0, 3)
    return np.ascontiguousarray(y.reshape(P, nch * S * width))


def _bf16(x):
    import ml_dtypes
    return np.ascontiguousarray(x).astype(ml_dtypes.bfloat16)


def prepare_inputs(query, key, value, Wq_w, Wq_b, Wk_w, Wk_b, Wv_w, Wv_b,
                   Wo_w, Wo_b):
    query = np.asarray(query, dtype=np.float32)
    key = np.asarray(key, dtype=np.float32)
    value = np.asarray(value, dtype=np.float32)
    w = {
        "wq": _bf16(_chunked(np.ascontiguousarray(np.asarray(Wq_w, np.float32).T))),
        "wk": _bf16(_chunked(np.ascontiguousarray(np.asarray(Wk_w, np.float32).T))),
        "wv": _bf16(_chunked(np.ascontiguousarray(np.asarray(Wv_w, np.float32).T))),
        "wo": _bf16(_chunked(np.ascontiguousarray(np.asarray(Wo_w, np.float32).T))),
    }
    biases = {"bq": np.asarray(Wq_b, np.float32), "bk": np.asarray(Wk_b, np.float32),
              "bv": np.asarray(Wv_b, np.float32), "bo": np.asarray(Wo_b, np.float32)}
    has_bias = {nm: bool(np.any(b)) for nm, b in biases.items()}

    qT = [_bf16(_chunked(np.ascontiguousarray(query[b].T))) for b in range(B)]

    in_maps = []
    for c in range(N_CORES):
        b, r = c // (N_CORES // B), c % (N_CORES // B)
        sl = slice(r * KSH, (r + 1) * KSH)
        m = {
            "qT": qT[b],
            "kT": _bf16(_part_major(np.ascontiguousarray(key[b, sl, :].T))),
            "vT": _bf16(_part_major(np.ascontiguousarray(value[b, sl, :].T))),
            **w,
        }
        for nm, hb in has_bias.items():
            if hb:
                m[nm] = biases[nm].reshape(1, D)
        in_maps.append(m)
    return in_maps, has_bias


def gather_output(results):
    out = np.empty((B, L, D), dtype=np.float32)
    for c in range(N_CORES):
        b, r = c // (N_CORES // B), c % (N_CORES // B)
        oT = results[c]["outT"].reshape(P, S, NQB, QS)
        for qb in range(NQB):
            blk = oT[:, :, qb, :].transpose(1, 0, 2).reshape(D, QS)
            q0 = qb * QB + r * QS
            out[b, q0:q0 + QS, :] = blk.T
    return out


def kernel(**inputs) -> np.ndarray:
    in_maps, has_bias = prepare_inputs(**inputs)
    nc = _get_program(has_bias)
    res = run_bass_kernel_spmd(nc, in_maps, list(range(N_CORES)))
    return gather_output(res.results)


# revision 16
# speedup vs baseline: 1.5025x; 1.4223x over previous
"""CategoryAttention (softmax over heads axis) on 8 Trainium2 cores.

Sharding v4 (2D q x k grid, no collectives): core c handles batch
b=c//4, q-half qh=(c%4)//2 (rows [qh*1024, +1024)) and k-half kh=c%2
(k-rows [kh*1024, +1024)) of its batch. Projections are sharded with
the grid (Q rows for the q-half, K/V rows for the k-half -> only 2x
redundancy, PE total ~191us/core vs 226 replicated). Softmax over the
HEAD axis is local per (k,q), so the k-split needs no communication;
each core emits a PARTIAL output (its k-half's AV context pushed
through the output projection, which is linear) in f32, and the HOST
sums the two k-half partials per (b, q-half) during gather/unshard.
Collectives were measured at ~58us/MB on this fabric (4-rank RS) --
host-side reduction of 4MB partials is strictly better.

Sweep: 16 units = 2 q-blocks x 8 k-tiles, each [128k x 16h x 512q]:
- ACT (~9.9us/unit wall): 8 exp activates + AV round-0 psum drains
- DVE (~10.9us/unit wall): head-sum tree, reciprocal (direct bf16),
  normalize (full-16-head broadcast multiply -- the known 2x path),
  AV round-1 accumulate adds
- GPSIMD: NOTHING (gpsimd tensor ops poison concurrent DVE throughput
  ~4x -- measured)
- PE (~8.5us/unit): energy (row-packed head pairs), AV (2-head
  row-packed tiles, 4-kt psum accumulation), K/Q/V/out projections
  interleaved as fillers between exp groups

psum banks: energy/proj/out 2x2 + av 3x1 = 7.
"""

import numpy as np
from contextlib import ExitStack

import concourse.bass as bass
import concourse.tile as tile
from concourse import bacc, mybir
from concourse.bass_utils import run_bass_kernel_spmd

F32 = mybir.dt.float32
BF16 = mybir.dt.bfloat16

N_CORES = 8
P = 128
D = 1024          # d_model
S = D // P        # 8 subtiles of the contraction dim
H = 16            # heads
HD = 64           # head dim
B = 2
L = 2048
LQ = L // 2             # 1024 q rows per core
NQB = 2                 # q blocks per core
QB = 512                # q rows per block/unit
LK = L // 2             # 1024 k rows per core
KTS = 128
NKT = LK // KTS         # 8 k-tiles
SCALE = 1.0 / np.sqrt(HD)

import os
WARM0 = int(os.environ.get("WARM0", "8"))
RECIP_BF16 = int(os.environ.get("RECIP_BF16", "0"))
AV_ACT_DRAIN = int(os.environ.get("AV_ACT_DRAIN", "1"))


def _build(has_bias):
    nc = bacc.Bacc("TRN2", target_bir_lowering=False, debug=False, num_devices=1)

    def din(name, shape, dt):
        return nc.dram_tensor(name, shape, dt, kind="ExternalInput").ap()

    qT_d = din("qT", (P, NQB * S * QB), BF16)   # q-half Q^T, chunked per block
    kT_d = din("kT", (P, S * LK), BF16)         # k-half input K^T
    vT_d = din("vT", (P, S * LK), BF16)
    wq_d = din("wq", (P, 2 * S * 512), BF16)
    wk_d = din("wk", (P, 2 * S * 512), BF16)
    wv_d = din("wv", (P, 2 * S * 512), BF16)
    wo_d = din("wo", (P, 2 * S * 512), BF16)
    bias_d = {}
    for nm in ("bq", "bk", "bv", "bo"):
        if has_bias[nm]:
            bias_d[nm] = din(nm, (1, D), F32)
    outT_d = nc.dram_tensor("outT", (P, S * LQ), F32, kind="ExternalOutput").ap()

    qT_ap = qT_d.rearrange("p (c s q) -> p c s q", c=NQB, s=S)
    kT_ap = kT_d.rearrange("p (s k) -> p s k", s=S)
    vT_ap = vT_d.rearrange("p (s k) -> p s k", s=S)
    wq_ap = wq_d.rearrange("p (h s o) -> p h s o", h=2, s=S)
    wk_ap = wk_d.rearrange("p (h s o) -> p h s o", h=2, s=S)
    wv_ap = wv_d.rearrange("p (h s o) -> p h s o", h=2, s=S)
    wo_ap = wo_d.rearrange("p (h s o) -> p h s o", h=2, s=S)
    outT_flat = outT_d.rearrange("p (j cq) -> p j cq", j=S)

    with tile.TileContext(nc) as tc, ExitStack() as ctx:
        # ---- persistent data tiles ----
        qt_pool = ctx.enter_context(tc.tile_pool(name="QT", bufs=1))
        kt_pool = ctx.enter_context(tc.tile_pool(name="KT", bufs=1))
        v_pool = ctx.enter_context(tc.tile_pool(name="V", bufs=1))
        QT_sb = qt_pool.tile([P, NQB, S, QB], BF16)
        KT_sb = kt_pool.tile([P, S, LK], BF16)
        V_sb = v_pool.tile([P, NKT, 2, 512], BF16)

        any_bias = any(has_bias.values())
        bias_t = {}
        ones_t = None
        if any_bias:
            cpool = ctx.enter_context(tc.tile_pool(name="const", bufs=1))
            ones_t = cpool.tile([1, 512], F32, tag="ones")
            nc.vector.memset(ones_t[:], 1.0)
            for nm, d_ap in bias_d.items():
                t = cpool.tile([1, D], F32, tag=f"bias_{nm}")
                nc.sync.dma_start(t[:], d_ap)
                bias_t[nm] = t

        def bias_mm(ps_t, bias_name, o0, n_sz, o_on_partitions):
            if o_on_partitions:
                nc.tensor.matmul(ps_t, lhsT=bias_t[bias_name][0:1, o0:o0 + P],
                                 rhs=ones_t[0:1, :n_sz], start=False, stop=True)
            else:
                nc.tensor.matmul(ps_t, lhsT=ones_t[0:1, 0:P],
                                 rhs=bias_t[bias_name][0:1, o0:o0 + n_sz],
                                 start=False, stop=True)

        # psum: energy/proj/out 2x2 banks + av 3x1 = 7 of 8
        e_psum = ctx.enter_context(tc.tile_pool(name="epsum", bufs=2, space="PSUM"))
        av_psum = ctx.enter_context(tc.tile_pool(name="avpsum", bufs=3, space="PSUM"))

        # one 8KB/part buffer chained kin -> vin -> qin(qb1); qin(qb0)
        # gets its own buffer (lifetimes overlap kin's).
        in_pool = ctx.enter_context(tc.tile_pool(name="instream", bufs=1))
        kin = in_pool.tile([P, S, LK], BF16, tag="ab", name="kint")
        nc.sync.dma_start(kin[:], kT_ap)
        vin_h = []

        def dma_vin():
            t = in_pool.tile([P, S, LK], BF16, tag="ab", name="vint")
            nc.sync.dma_start(t[:], vT_ap)
            vin_h.append(t)
        qin_pool = ctx.enter_context(tc.tile_pool(name="qin", bufs=1))
        pw_pool = ctx.enter_context(tc.tile_pool(name="pw", bufs=1))
        wk_h, wv_h, wq_h = [], [], []
        for wh in range(2):
            t = pw_pool.tile([P, S, 512], BF16, tag=f"pw{wh}", name="wkt")
            nc.sync.dma_start(t[:], wk_ap[:, wh])
            wk_h.append(t)
        for wh in range(2):
            t = pw_pool.tile([P, S, 512], BF16, tag=f"wq{wh}", name="wqt")
            nc.sync.dma_start(t[:], wq_ap[:, wh])
            wq_h.append(t)

        def dma_wv():
            for wh in range(2):
                t = pw_pool.tile([P, S, 512], BF16, tag=f"pw{wh}", name="wvt")
                nc.sync.dma_start(t[:], wv_ap[:, wh])
                wv_h.append(t)

        # PE warm-up during initial input DMAs (HAM clock-gate ramp).
        warm_pool = ctx.enter_context(tc.tile_pool(name="warm", bufs=1))
        wrm = warm_pool.tile([P, 128], BF16, tag="wrm")
        nc.vector.memset(wrm[:], 0.0)
        for _ in range(WARM0):
            wps = e_psum.tile([P, 2, QB], F32, tag="e")
            for hh in range(2):
                nc.tensor.matmul(wps[:, hh, 0:128], lhsT=wrm[:, 0:P],
                                 rhs=wrm[:, :], start=True, stop=True)

        # ---- projection pieces (emitted as fillers) ----
        def k_proj_pair(jp):
            """K^T o-tiles 2jp,2jp+1 (two 512-col half passes)."""
            for kh in range(2):
                k0 = kh * 512
                ps = e_psum.tile([P, 2, 512], F32, tag="e")
                for hh in range(2):
                    j = 2 * jp + hh
                    w_t = wk_h[j // 4]
                    jl = j % 4
                    for s in range(S):
                        nc.tensor.matmul(
                            ps[:, hh, :],
                            lhsT=w_t[:, s, jl * P:(jl + 1) * P],
                            rhs=kin[:, s, k0:k0 + 512],
                            start=(s == 0),
                            stop=(s == S - 1 and not has_bias["bk"]),
                        )
                    if has_bias["bk"]:
                        bias_mm(ps[:, hh, :], "bk", j * P, 512, True)
                nc.scalar.copy(KT_sb[:, 2 * jp:2 * jp + 2, k0:k0 + 512], ps[:])

        def v_proj_group(kt4):
            """V rows for k-subtile kt4 (128 rows, all 1024 cols)."""
            ps = e_psum.tile([P, 2, 512], F32, tag="e")
            for t in range(2):
                for s in range(S):
                    nc.tensor.matmul(
                        ps[:, t, :],
                        lhsT=vin_h[0][:, s, kt4 * P:(kt4 + 1) * P],
                        rhs=wv_h[t][:, s, :],
                        start=(s == 0),
                        stop=(s == S - 1 and not has_bias["bv"]),
                    )
                if has_bias["bv"]:
                    bias_mm(ps[:, t, :], "bv", t * 512, 512, False)
            nc.scalar.copy(V_sb[:, kt4, :, :], ps[:])

        qctx = {}

        def q_proj_start(qb):
            pool, tg = (qin_pool, "qin") if qb == 0 else (in_pool, "ab")
            qin = pool.tile([P, S, QB], BF16, tag=tg, name="qint")
            nc.sync.dma_start(qin[:], qT_ap[:, qb])
            qctx[qb] = qin

        def q_proj_pair(qb, jp):
            qin = qctx[qb]
            ps = e_psum.tile([P, 2, QB], F32, tag="e")
            for hh in range(2):
                j = 2 * jp + hh
                w_t = wq_h[j // 4]
                jl = j % 4
                for s in range(S):
                    nc.tensor.matmul(
                        ps[:, hh, :],
                        lhsT=w_t[:, s, jl * P:(jl + 1) * P],
                        rhs=qin[:, s, :],
                        start=(s == 0),
                        stop=(s == S - 1 and not has_bias["bq"]),
                    )
                if has_bias["bq"]:
                    bias_mm(ps[:, hh, :], "bq", j * P, QB, True)
            nc.scalar.copy(QT_sb[:, qb, 2 * jp:2 * jp + 2, :], ps[:])

        # ---- attention-era pools ----
        attn_pool = ctx.enter_context(tc.tile_pool(name="attn", bufs=5))
        tree_pool = ctx.enter_context(tc.tile_pool(name="tree", bufs=1))
        den_pool = ctx.enter_context(tc.tile_pool(name="den", bufs=1))
        rb_pool = ctx.enter_context(tc.tile_pool(name="rb", bufs=1))
        ctx_pool = ctx.enter_context(tc.tile_pool(name="ctx", bufs=1))
        osb_pool = ctx.enter_context(tc.tile_pool(name="osb", bufs=2))

        # ---------------- attention unit ----------------
        def softmax_unit(qb, kt, fillers=()):
            """energy -> exp -> head-sum -> reciprocal -> normalize.
            fillers: PE work thunks interleaved after exp groups."""
            attn_t = attn_pool.tile([P, H, QB], BF16, tag="attn")
            fi = 0
            for g in range(8):
                eps = e_psum.tile([P, 2, QB], F32, tag="e")
                for hh in range(2):
                    p0 = HD * hh
                    nc.tensor.matmul(
                        eps[:, hh, :],
                        lhsT=KT_sb[p0:p0 + HD, g, kt * KTS:(kt + 1) * KTS],
                        rhs=QT_sb[p0:p0 + HD, qb, g, :],
                        start=True,
                        stop=True,
                    )
                nc.scalar.activation(attn_t[:, g * 2:(g + 1) * 2, :], eps[:],
                                     mybir.ActivationFunctionType.Exp,
                                     scale=float(SCALE))
                while fi * 8 < (g + 1) * len(fillers):
                    fillers[fi]()
                    fi += 1
            t1 = tree_pool.tile([P, 4, QB], BF16, tag="t1")
            with nc.allow_low_precision(reason="bf16 head-sum tree"):
                nc.vector.tensor_add(t1[:], attn_t[:, 0:4, :], attn_t[:, 4:8, :])
                nc.vector.tensor_add(t1[:], t1[:], attn_t[:, 8:12, :])
                nc.vector.tensor_add(t1[:], t1[:], attn_t[:, 12:16, :])
                nc.vector.tensor_add(t1[:, 0:2, :], t1[:, 0:2, :], t1[:, 2:4, :])
            den = den_pool.tile([P, QB], F32, tag="den")
            nc.vector.tensor_add(den[:], t1[:, 0, :], t1[:, 1, :])
            rb = rb_pool.tile([P, QB], BF16, tag="rb")
            with nc.allow_low_precision(reason="bf16 reciprocal"):
                if RECIP_BF16:
                    nc.vector.reciprocal_approx_fast(rb[:], den[:])
                else:
                    r32 = den_pool.tile([P, QB], F32, tag="r", name="r32")
                    nc.vector.reciprocal_approx_fast(r32[:], den[:])
                    nc.vector.tensor_copy(rb[:], r32[:])
            nc.vector.tensor_mul(attn_t[:], attn_t[:],
                                 rb[:, None, :].to_broadcast((P, H, QB)))
            return attn_t

        ctx_tiles = {}

        def av_pair(qb, rnd, u, attn_list):
            """Heads 2u,2u+1 x 4 k-tiles (round rnd) -> [128,512] psum tile
            (head parity on partition halves) -> ctx j-subtile u."""
            if rnd == 0 and u == 0:
                ctx_tiles[qb] = ctx_pool.tile([P, S, QB], BF16, tag="ctx",
                                              name="ctxp")
            avp = av_psum.tile([P, QB], F32, tag="av")
            for ki in range(4):
                kt = 4 * rnd + ki
                for hh in range(2):
                    h = 2 * u + hh
                    p0 = HD * hh
                    nc.tensor.matmul(
                        avp[p0:p0 + HD, :],
                        lhsT=V_sb[:, kt, h // 8, (h % 8) * HD:(h % 8 + 1) * HD],
                        rhs=attn_list[ki][:, h, :],
                        start=(ki == 0),
                        stop=(ki == 3),
                    )
            with nc.allow_low_precision(reason="bf16 ctx partial"):
                if rnd == 0:
                    if AV_ACT_DRAIN:
                        nc.scalar.copy(ctx_tiles[qb][:, u, :], avp[:])
                    else:
                        nc.vector.tensor_copy(ctx_tiles[qb][:, u, :], avp[:])
                else:
                    nc.vector.tensor_add(ctx_tiles[qb][:, u, :],
                                         ctx_tiles[qb][:, u, :], avp[:])

        wo_tiles = []

        def dma_wo():
            for wh in range(2):
                t = pw_pool.tile([P, S, 512], BF16, tag=f"pw{wh}", name="wot")
                nc.sync.dma_start(t[:], wo_ap[:, wh])
                wo_tiles.append(t)

        def out_proj_quarter(qb, j2):
            """Output projection o-tiles 2*j2, 2*j2+1 for q-block qb."""
            woh = wo_tiles[j2 // 2]
            po = e_psum.tile([P, 2, QB], F32, tag="e")
            for jj in range(2):
                j = j2 * 2 + jj
                jl = (j2 % 2) * 2 + jj
                for s in range(S):
                    nc.tensor.matmul(
                        po[:, jj, :],
                        lhsT=woh[:, s, jl * P:(jl + 1) * P],
                        rhs=ctx_tiles[qb][:, s, :],
                        start=(s == 0),
                        stop=(s == S - 1 and not has_bias["bo"]),
                    )
                if has_bias["bo"]:
                    bias_mm(po[:, jj, :], "bo", j * P, QB, True)
            for jj in range(2):
                osb = osb_pool.tile([P, QB], F32, tag="osb")
                nc.scalar.copy(osb[:], po[:, jj, :])
                nc.sync.dma_start(outT_flat[:, 2 * j2 + jj,
                                           qb * QB:(qb + 1) * QB], osb[:])

        # ---------------- sweep ----------------
        # 16 units (qb, kt). Filler schedule (PE work between exp groups):
        #  qb0-kt0: K jp2/jp3 + Q0 jp2/jp3 (jp0/jp1 emitted just before)
        #  qb0-kt1..kt3: V groups 0..7
        #  qb0-kt4: av(qb0, round0)  [kt0-3 attn]
        #  qb0-kt5/6: Q1 jp0..jp3;  qb0-kt7: wo DMA
        #  qb1-kt0: av(qb0, round1) [kt4-7 attn] -> ctx(qb0) complete
        #  qb1-kt1/2: out_proj(qb0) quarters
        #  qb1-kt4: av(qb1, round0); tail: av(qb1, round1) + out_proj(qb1)
        q_proj_start(0)
        k_proj_pair(0)
        q_proj_pair(0, 0)
        k_proj_pair(1)
        q_proj_pair(0, 1)

        attn_tiles = {}
        for qb in range(NQB):
            attn_tiles[qb] = []
            for kt in range(NKT):
                fillers = []
                if qb == 0 and kt == 0:
                    # all four pieces must land before energy groups 4/6
                    # read the K/Q tiles they produce -> fire at g0..g3
                    noop = lambda: None
                    fillers = [
                        lambda: k_proj_pair(2),
                        lambda: q_proj_pair(0, 2),
                        lambda: k_proj_pair(3),
                        lambda: q_proj_pair(0, 3),
                        noop, noop, noop, noop,
                    ]
                elif qb == 0 and kt in (1, 2, 3):
                    g0 = 3 * (kt - 1)
                    n = 2 if kt == 3 else 3
                    fillers = [(lambda g=g: v_proj_group(g))
                               for g in range(g0, g0 + n)]
                elif kt == 4:
                    prev = attn_tiles[qb][0:4]
                    fillers = [
                        (lambda u=u, p=prev, q=qb: av_pair(q, 0, u, p))
                        for u in range(8)
                    ]
                elif qb == 0 and kt == 5:
                    fillers = [lambda: q_proj_pair(1, 0), lambda: q_proj_pair(1, 1)]
                elif qb == 0 and kt == 6:
                    fillers = [lambda: q_proj_pair(1, 2), lambda: q_proj_pair(1, 3)]
                elif qb == 1 and kt == 0:
                    prevu = attn_tiles[0][4:8]
                    fillers = [
                        (lambda u=u, p=prevu: av_pair(0, 1, u, p))
                        for u in range(8)
                    ]
                elif qb == 1 and kt in (1, 2):
                    j0 = 2 * (kt - 1)
                    fillers = [(lambda j=j: out_proj_quarter(0, j))
                               for j in (j0, j0 + 1)]
                attn_tiles[qb].append(softmax_unit(qb, kt, fillers))
                if qb == 0 and kt == 0:
                    dma_vin()
                    dma_wv()
                if qb == 0 and kt == 4:
                    q_proj_start(1)
                if qb == 0 and kt == 7:
                    dma_wo()

        # tail
        prevu = attn_tiles[1][4:8]
        for u in range(8):
            av_pair(1, 1, u, prevu)
        for j2 in range(4):
            out_proj_quarter(1, j2)

    nc.compile()
    return nc


_cache = {}


def _get_program(has_bias):
    key = (RECIP_BF16, AV_ACT_DRAIN, tuple(sorted(has_bias.items())))
    if key not in _cache:
        _cache[key] = _build(has_bias)
    return _cache[key]


def _part_major(x):
    n = x.shape[1]
    return np.ascontiguousarray(
        x.reshape(S, P, n).transpose(1, 0, 2).reshape(P, S * n))


def _chunked(x, width=512):
    """[D, N] -> [P, N//width, S, width] per-chunk contiguous layout."""
    n = x.shape[1]
    nch = n // width
    y = x.reshape(S, P, nch, width).transpose(1, 2, 0, 3)
    return np.ascontiguousarray(y.reshape(P, nch * S * width))


def _bf16(x):
    import ml_dtypes
    return np.ascontiguousarray(x).astype(ml_dtypes.bfloat16)


def prepare_inputs(query, key, value, Wq_w, Wq_b, Wk_w, Wk_b, Wv_w, Wv_b,
                   Wo_w, Wo_b):
    query = np.asarray(query, dtype=np.float32)
    key = np.asarray(key, dtype=np.float32)
    value = np.asarray(value, dtype=np.float32)
    w = {
        "wq": _bf16(_chunked(np.ascontiguousarray(np.asarray(Wq_w, np.float32).T))),
        "wk": _bf16(_chunked(np.ascontiguousarray(np.asarray(Wk_w, np.float32).T))),
        "wv": _bf16(_chunked(np.ascontiguousarray(np.asarray(Wv_w, np.float32).T))),
        "wo": _bf16(_chunked(np.ascontiguousarray(np.asarray(Wo_w, np.float32).T))),
    }
    biases = {"bq": np.asarray(Wq_b, np.float32), "bk": np.asarray(Wk_b, np.float32),
              "bv": np.asarray(Wv_b, np.float32), "bo": np.asarray(Wo_b, np.float32)}
    has_bias = {nm: bool(np.any(b)) for nm, b in biases.items()}

    in_maps = []
    for c in range(N_CORES):
        b = c // 4
        qh = (c % 4) // 2
        kh = c % 2
        qs = slice(qh * LQ, (qh + 1) * LQ)
        ks = slice(kh * LK, (kh + 1) * LK)
        m = {
            "qT": _bf16(_chunked(np.ascontiguousarray(query[b, qs, :].T))),
            "kT": _bf16(_part_major(np.ascontiguousarray(key[b, ks, :].T))),
            "vT": _bf16(_part_major(np.ascontiguousarray(value[b, ks, :].T))),
            **w,
        }
        for nm, hb in has_bias.items():
            if hb:
                m[nm] = biases[nm].reshape(1, D)
        in_maps.append(m)
    return in_maps, has_bias


def gather_output(results):
    out = np.zeros((B, L, D), dtype=np.float32)
    for c in range(N_CORES):
        b = c // 4
        qh = (c % 4) // 2
        oT = results[c]["outT"].reshape(P, S, LQ).transpose(1, 0, 2).reshape(D, LQ)
        out[b, qh * LQ:(qh + 1) * LQ, :] += oT.T
    return out


def kernel(**inputs) -> np.ndarray:
    in_maps, has_bias = prepare_inputs(**inputs)
    nc = _get_program(has_bias)
    res = run_bass_kernel_spmd(nc, in_maps, list(range(N_CORES)))
    return gather_output(res.results)


# revision 17
# speedup vs baseline: 1.5450x; 1.0283x over previous
"""CategoryAttention (softmax over heads axis) on 8 Trainium2 cores.

Sharding v4 (2D q x k grid, no collectives): core c handles batch
b=c//4, q-half qh=(c%4)//2 (rows [qh*1024, +1024)) and k-half kh=c%2
(k-rows [kh*1024, +1024)) of its batch. Projections are sharded with
the grid (Q rows for the q-half, K/V rows for the k-half -> only 2x
redundancy, PE total ~191us/core vs 226 replicated). Softmax over the
HEAD axis is local per (k,q), so the k-split needs no communication;
each core emits a PARTIAL output (its k-half's AV context pushed
through the output projection, which is linear) in f32, and the HOST
sums the two k-half partials per (b, q-half) during gather/unshard.
Collectives were measured at ~58us/MB on this fabric (4-rank RS) --
host-side reduction of 4MB partials is strictly better.

Sweep: 16 units = 2 q-blocks x 8 k-tiles, each [128k x 16h x 512q]:
- ACT (~9.9us/unit wall): 8 exp activates + AV round-0 psum drains
- DVE (~10.9us/unit wall): head-sum tree, reciprocal (direct bf16),
  normalize (full-16-head broadcast multiply -- the known 2x path),
  AV round-1 accumulate adds
- GPSIMD: NOTHING (gpsimd tensor ops poison concurrent DVE throughput
  ~4x -- measured)
- PE (~8.5us/unit): energy (row-packed head pairs), AV (2-head
  row-packed tiles, 4-kt psum accumulation), K/Q/V/out projections
  interleaved as fillers between exp groups

psum banks: energy/proj/out 2x2 + av 3x1 = 7.
"""

import numpy as np
from contextlib import ExitStack

import concourse.bass as bass
import concourse.tile as tile
from concourse import bacc, mybir
from concourse.bass_utils import run_bass_kernel_spmd

F32 = mybir.dt.float32
BF16 = mybir.dt.bfloat16

N_CORES = 8
P = 128
D = 1024          # d_model
S = D // P        # 8 subtiles of the contraction dim
H = 16            # heads
HD = 64           # head dim
B = 2
L = 2048
LQ = L // 2             # 1024 q rows per core
NQB = 2                 # q blocks per core
QB = 512                # q rows per block/unit
LK = L // 2             # 1024 k rows per core
KTS = 128
NKT = LK // KTS         # 8 k-tiles
SCALE = 1.0 / np.sqrt(HD)

import os
WARM0 = int(os.environ.get("WARM0", "48"))
RECIP_BF16 = int(os.environ.get("RECIP_BF16", "0"))
AV_ACT_DRAIN = int(os.environ.get("AV_ACT_DRAIN", "1"))


def _build(has_bias):
    nc = bacc.Bacc("TRN2", target_bir_lowering=False, debug=False, num_devices=1)

    def din(name, shape, dt):
        return nc.dram_tensor(name, shape, dt, kind="ExternalInput").ap()

    qT_d = din("qT", (P, NQB * S * QB), BF16)   # q-half Q^T, chunked per block
    kT_d = din("kT", (P, S * LK), BF16)         # k-half input K^T
    vT_d = din("vT", (P, S * LK), BF16)
    wq_d = din("wq", (P, 2 * S * 512), BF16)
    wk_d = din("wk", (P, 2 * S * 512), BF16)
    wv_d = din("wv", (P, 2 * S * 512), BF16)
    wo_d = din("wo", (P, 2 * S * 512), BF16)
    bias_d = {}
    for nm in ("bq", "bk", "bv", "bo"):
        if has_bias[nm]:
            bias_d[nm] = din(nm, (1, D), F32)
    outT_d = nc.dram_tensor("outT", (P, S * LQ), F32, kind="ExternalOutput").ap()

    qT_ap = qT_d.rearrange("p (c s q) -> p c s q", c=NQB, s=S)
    kT_ap = kT_d.rearrange("p (s k) -> p s k", s=S)
    vT_ap = vT_d.rearrange("p (s k) -> p s k", s=S)
    wq_ap = wq_d.rearrange("p (h s o) -> p h s o", h=2, s=S)
    wk_ap = wk_d.rearrange("p (h s o) -> p h s o", h=2, s=S)
    wv_ap = wv_d.rearrange("p (h s o) -> p h s o", h=2, s=S)
    wo_ap = wo_d.rearrange("p (h s o) -> p h s o", h=2, s=S)
    outT_flat = outT_d.rearrange("p (j cq) -> p j cq", j=S)

    with tile.TileContext(nc) as tc, ExitStack() as ctx:
        # ---- persistent data tiles ----
        qt_pool = ctx.enter_context(tc.tile_pool(name="QT", bufs=1))
        kt_pool = ctx.enter_context(tc.tile_pool(name="KT", bufs=1))
        v_pool = ctx.enter_context(tc.tile_pool(name="V", bufs=1))
        QT_sb = qt_pool.tile([P, NQB, S, QB], BF16)
        KT_sb = kt_pool.tile([P, S, LK], BF16)
        V_sb = v_pool.tile([P, NKT, 2, 512], BF16)

        any_bias = any(has_bias.values())
        bias_t = {}
        ones_t = None
        if any_bias:
            cpool = ctx.enter_context(tc.tile_pool(name="const", bufs=1))
            ones_t = cpool.tile([1, 512], F32, tag="ones")
            nc.vector.memset(ones_t[:], 1.0)
            for nm, d_ap in bias_d.items():
                t = cpool.tile([1, D], F32, tag=f"bias_{nm}")
                nc.sync.dma_start(t[:], d_ap)
                bias_t[nm] = t

        def bias_mm(ps_t, bias_name, o0, n_sz, o_on_partitions):
            if o_on_partitions:
                nc.tensor.matmul(ps_t, lhsT=bias_t[bias_name][0:1, o0:o0 + P],
                                 rhs=ones_t[0:1, :n_sz], start=False, stop=True)
            else:
                nc.tensor.matmul(ps_t, lhsT=ones_t[0:1, 0:P],
                                 rhs=bias_t[bias_name][0:1, o0:o0 + n_sz],
                                 start=False, stop=True)

        # psum: energy/proj/out 2x2 banks + av 3x1 = 7 of 8
        e_psum = ctx.enter_context(tc.tile_pool(name="epsum", bufs=2, space="PSUM"))
        av_psum = ctx.enter_context(tc.tile_pool(name="avpsum", bufs=4, space="PSUM"))

        # one 8KB/part buffer chained kin -> vin -> qin(qb1); qin(qb0)
        # gets its own buffer (lifetimes overlap kin's).
        in_pool = ctx.enter_context(tc.tile_pool(name="instream", bufs=1))
        kin = in_pool.tile([P, S, LK], BF16, tag="ab", name="kint")
        nc.sync.dma_start(kin[:], kT_ap)
        vin_h = []

        def dma_vin():
            t = in_pool.tile([P, S, LK], BF16, tag="ab", name="vint")
            nc.sync.dma_start(t[:], vT_ap)
            vin_h.append(t)
        qin_pool = ctx.enter_context(tc.tile_pool(name="qin", bufs=1))
        pw_pool = ctx.enter_context(tc.tile_pool(name="pw", bufs=1))
        wk_h, wv_h, wq_h = [], [], []
        for wh in range(2):
            t = pw_pool.tile([P, S, 512], BF16, tag=f"pw{wh}", name="wkt")
            nc.sync.dma_start(t[:], wk_ap[:, wh])
            wk_h.append(t)
        for wh in range(2):
            t = pw_pool.tile([P, S, 512], BF16, tag=f"wq{wh}", name="wqt")
            nc.sync.dma_start(t[:], wq_ap[:, wh])
            wq_h.append(t)

        def dma_wv():
            for wh in range(2):
                t = pw_pool.tile([P, S, 512], BF16, tag=f"pw{wh}", name="wvt")
                nc.sync.dma_start(t[:], wv_ap[:, wh])
                wv_h.append(t)

        # PE warm-up during initial input DMAs (HAM clock-gate ramp).
        warm_pool = ctx.enter_context(tc.tile_pool(name="warm", bufs=1))
        wrm = warm_pool.tile([P, 128], BF16, tag="wrm")
        nc.vector.memset(wrm[:], 0.0)
        for _ in range(WARM0):
            wps = e_psum.tile([P, 2, QB], F32, tag="e")
            for hh in range(2):
                nc.tensor.matmul(wps[:, hh, 0:128], lhsT=wrm[:, 0:P],
                                 rhs=wrm[:, :], start=True, stop=True)

        # ---- projection pieces (emitted as fillers) ----
        def k_proj_pair(jp):
            """K^T o-tiles 2jp,2jp+1 (two 512-col half passes)."""
            for kh in range(2):
                k0 = kh * 512
                ps = e_psum.tile([P, 2, 512], F32, tag="e")
                for hh in range(2):
                    j = 2 * jp + hh
                    w_t = wk_h[j // 4]
                    jl = j % 4
                    for s in range(S):
                        nc.tensor.matmul(
                            ps[:, hh, :],
                            lhsT=w_t[:, s, jl * P:(jl + 1) * P],
                            rhs=kin[:, s, k0:k0 + 512],
                            start=(s == 0),
                            stop=(s == S - 1 and not has_bias["bk"]),
                        )
                    if has_bias["bk"]:
                        bias_mm(ps[:, hh, :], "bk", j * P, 512, True)
                nc.scalar.copy(KT_sb[:, 2 * jp:2 * jp + 2, k0:k0 + 512], ps[:])

        def v_proj_group(kt4):
            """V rows for k-subtile kt4 (128 rows, all 1024 cols)."""
            ps = e_psum.tile([P, 2, 512], F32, tag="e")
            for t in range(2):
                for s in range(S):
                    nc.tensor.matmul(
                        ps[:, t, :],
                        lhsT=vin_h[0][:, s, kt4 * P:(kt4 + 1) * P],
                        rhs=wv_h[t][:, s, :],
                        start=(s == 0),
                        stop=(s == S - 1 and not has_bias["bv"]),
                    )
                if has_bias["bv"]:
                    bias_mm(ps[:, t, :], "bv", t * 512, 512, False)
            nc.scalar.copy(V_sb[:, kt4, :, :], ps[:])

        qctx = {}

        def q_proj_start(qb):
            pool, tg = (qin_pool, "qin") if qb == 0 else (in_pool, "ab")
            qin = pool.tile([P, S, QB], BF16, tag=tg, name="qint")
            nc.sync.dma_start(qin[:], qT_ap[:, qb])
            qctx[qb] = qin

        def q_proj_pair(qb, jp):
            qin = qctx[qb]
            ps = e_psum.tile([P, 2, QB], F32, tag="e")
            for hh in range(2):
                j = 2 * jp + hh
                w_t = wq_h[j // 4]
                jl = j % 4
                for s in range(S):
                    nc.tensor.matmul(
                        ps[:, hh, :],
                        lhsT=w_t[:, s, jl * P:(jl + 1) * P],
                        rhs=qin[:, s, :],
                        start=(s == 0),
                        stop=(s == S - 1 and not has_bias["bq"]),
                    )
                if has_bias["bq"]:
                    bias_mm(ps[:, hh, :], "bq", j * P, QB, True)
            nc.scalar.copy(QT_sb[:, qb, 2 * jp:2 * jp + 2, :], ps[:])

        # ---- attention-era pools ----
        attn_pool = ctx.enter_context(tc.tile_pool(name="attn", bufs=5))
        tree_pool = ctx.enter_context(tc.tile_pool(name="tree", bufs=1))
        den_pool = ctx.enter_context(tc.tile_pool(name="den", bufs=1))
        rb_pool = ctx.enter_context(tc.tile_pool(name="rb", bufs=1))
        ctx_pool = ctx.enter_context(tc.tile_pool(name="ctx", bufs=1))
        osb_pool = ctx.enter_context(tc.tile_pool(name="osb", bufs=2))

        # ---------------- attention unit ----------------
        def softmax_unit(qb, kt, fillers=()):
            """energy -> exp -> head-sum -> reciprocal -> normalize.
            fillers: PE work thunks interleaved after exp groups."""
            attn_t = attn_pool.tile([P, H, QB], BF16, tag="attn")
            fi = 0
            for g in range(8):
                eps = e_psum.tile([P, 2, QB], F32, tag="e")
                for hh in range(2):
                    p0 = HD * hh
                    nc.tensor.matmul(
                        eps[:, hh, :],
                        lhsT=KT_sb[p0:p0 + HD, g, kt * KTS:(kt + 1) * KTS],
                        rhs=QT_sb[p0:p0 + HD, qb, g, :],
                        start=True,
                        stop=True,
                    )
                nc.scalar.activation(attn_t[:, g * 2:(g + 1) * 2, :], eps[:],
                                     mybir.ActivationFunctionType.Exp,
                                     scale=float(SCALE))
                while fi * 8 < (g + 1) * len(fillers):
                    fillers[fi]()
                    fi += 1
            t1 = tree_pool.tile([P, 4, QB], BF16, tag="t1")
            with nc.allow_low_precision(reason="bf16 head-sum tree"):
                nc.vector.tensor_add(t1[:], attn_t[:, 0:4, :], attn_t[:, 4:8, :])
                nc.vector.tensor_add(t1[:], t1[:], attn_t[:, 8:12, :])
                nc.vector.tensor_add(t1[:], t1[:], attn_t[:, 12:16, :])
                nc.vector.tensor_add(t1[:, 0:2, :], t1[:, 0:2, :], t1[:, 2:4, :])
            den = den_pool.tile([P, QB], F32, tag="den")
            nc.vector.tensor_add(den[:], t1[:, 0, :], t1[:, 1, :])
            rb = rb_pool.tile([P, QB], BF16, tag="rb")
            with nc.allow_low_precision(reason="bf16 reciprocal"):
                if RECIP_BF16:
                    nc.vector.reciprocal_approx_fast(rb[:], den[:])
                else:
                    r32 = den_pool.tile([P, QB], F32, tag="r", name="r32")
                    nc.vector.reciprocal_approx_fast(r32[:], den[:])
                    nc.vector.tensor_copy(rb[:], r32[:])
            nc.vector.tensor_mul(attn_t[:], attn_t[:],
                                 rb[:, None, :].to_broadcast((P, H, QB)))
            return attn_t

        ctx_tiles = {}

        def av_pair(qb, kts, u, first):
            """Heads 2u,2u+1 x the given k-tiles -> [128,512] psum tile
            (head parity on partition halves) -> ctx j-subtile u."""
            if first and u == 0:
                ctx_tiles[qb] = ctx_pool.tile([P, S, QB], BF16, tag="ctx",
                                              name="ctxp")
            avp = av_psum.tile([P, QB], F32, tag="av")
            for ki, kt in enumerate(kts):
                for hh in range(2):
                    h = 2 * u + hh
                    p0 = HD * hh
                    nc.tensor.matmul(
                        avp[p0:p0 + HD, :],
                        lhsT=V_sb[:, kt, h // 8, (h % 8) * HD:(h % 8 + 1) * HD],
                        rhs=attn_tiles[qb][kt][:, h, :],
                        start=(ki == 0),
                        stop=(ki == len(kts) - 1),
                    )
            with nc.allow_low_precision(reason="bf16 ctx partial"):
                if first:
                    if AV_ACT_DRAIN:
                        nc.scalar.copy(ctx_tiles[qb][:, u, :], avp[:])
                    else:
                        nc.vector.tensor_copy(ctx_tiles[qb][:, u, :], avp[:])
                else:
                    nc.vector.tensor_add(ctx_tiles[qb][:, u, :],
                                         ctx_tiles[qb][:, u, :], avp[:])

        wo_tiles = []

        def dma_wo():
            for wh in range(2):
                t = pw_pool.tile([P, S, 512], BF16, tag=f"pw{wh}", name="wot")
                nc.sync.dma_start(t[:], wo_ap[:, wh])
                wo_tiles.append(t)

        def out_proj_quarter(qb, j2):
            """Output projection o-tiles 2*j2, 2*j2+1 for q-block qb."""
            woh = wo_tiles[j2 // 2]
            po = e_psum.tile([P, 2, QB], F32, tag="e")
            for jj in range(2):
                j = j2 * 2 + jj
                jl = (j2 % 2) * 2 + jj
                for s in range(S):
                    nc.tensor.matmul(
                        po[:, jj, :],
                        lhsT=woh[:, s, jl * P:(jl + 1) * P],
                        rhs=ctx_tiles[qb][:, s, :],
                        start=(s == 0),
                        stop=(s == S - 1 and not has_bias["bo"]),
                    )
                if has_bias["bo"]:
                    bias_mm(po[:, jj, :], "bo", j * P, QB, True)
            for jj in range(2):
                osb = osb_pool.tile([P, QB], F32, tag="osb")
                nc.scalar.copy(osb[:], po[:, jj, :])
                nc.sync.dma_start(outT_flat[:, 2 * j2 + jj,
                                           qb * QB:(qb + 1) * QB], osb[:])

        # ---------------- sweep ----------------
        # 16 units (qb, kt). Filler schedule (PE work between exp groups):
        #  qb0-kt0: K jp2/jp3 + Q0 jp2/jp3 (jp0/jp1 emitted just before)
        #  qb0-kt1..kt3: V groups 0..7
        #  qb0-kt4: av(qb0, round0)  [kt0-3 attn]
        #  qb0-kt5/6: Q1 jp0..jp3;  qb0-kt7: wo DMA
        #  qb1-kt0: av(qb0, round1) [kt4-7 attn] -> ctx(qb0) complete
        #  qb1-kt1/2: out_proj(qb0) quarters
        #  qb1-kt4: av(qb1, round0); tail: av(qb1, round1) + out_proj(qb1)
        q_proj_start(0)
        k_proj_pair(0)
        q_proj_pair(0, 0)
        k_proj_pair(1)
        q_proj_pair(0, 1)

        attn_tiles = {0: [], 1: []}
        noop = lambda: None
        for qb in range(NQB):
            for kt in range(NKT):
                fillers = []
                if qb == 0 and kt == 0:
                    # all four pieces must land before energy groups 4/6
                    # read the K/Q tiles they produce -> fire at g0..g3
                    fillers = [
                        lambda: k_proj_pair(2),
                        lambda: q_proj_pair(0, 2),
                        lambda: k_proj_pair(3),
                        lambda: q_proj_pair(0, 3),
                        noop, noop, noop, noop,
                    ]
                elif qb == 0 and kt in (1, 2, 3):
                    g0 = 3 * (kt - 1)
                    n = 2 if kt == 3 else 3
                    fillers = [(lambda g=g: v_proj_group(g))
                               for g in range(g0, g0 + n)]
                elif kt == 4:
                    fillers = [
                        (lambda u=u, q=qb: av_pair(q, [0, 1, 2, 3], u, True))
                        for u in range(8)
                    ]
                elif qb == 0 and kt == 5:
                    fillers = [lambda: q_proj_pair(1, 0), lambda: q_proj_pair(1, 1)]
                elif qb == 0 and kt == 6:
                    fillers = ([lambda: q_proj_pair(1, 2),
                                lambda: q_proj_pair(1, 3)] +
                               [(lambda u=u: av_pair(0, [4, 5], u, False))
                                for u in range(4)])
                elif qb == 1 and kt == 6:
                    fillers = [(lambda u=u: av_pair(1, [4, 5], u, False))
                               for u in range(4)]
                elif kt == 7:
                    fillers = [(lambda u=u, q=qb: av_pair(q, [4, 5], u, False))
                               for u in range(4, 8)]
                elif qb == 1 and kt == 0:
                    fillers = [(lambda u=u: av_pair(0, [6, 7], u, False))
                               for u in range(8)]
                elif qb == 1 and kt in (1, 2):
                    j0 = 2 * (kt - 1)
                    fillers = [(lambda j=j: out_proj_quarter(0, j))
                               for j in (j0, j0 + 1)]
                attn_tiles[qb].append(softmax_unit(qb, kt, fillers))
                if qb == 0 and kt == 0:
                    dma_vin()
                    dma_wv()
                if qb == 0 and kt == 4:
                    q_proj_start(1)
                if qb == 0 and kt == 7:
                    dma_wo()

        # tail: last 2-kt sub-round + output projection of qb1
        for u in range(8):
            av_pair(1, [6, 7], u, False)
        for j2 in range(4):
            out_proj_quarter(1, j2)

    nc.compile()
    return nc


_cache = {}


def _get_program(has_bias):
    key = (RECIP_BF16, AV_ACT_DRAIN, tuple(sorted(has_bias.items())))
    if key not in _cache:
        _cache[key] = _build(has_bias)
    return _cache[key]


def _part_major(x):
    n = x.shape[1]
    return np.ascontiguousarray(
        x.reshape(S, P, n).transpose(1, 0, 2).reshape(P, S * n))


def _chunked(x, width=512):
    """[D, N] -> [P, N//width, S, width] per-chunk contiguous layout."""
    n = x.shape[1]
    nch = n // width
    y = x.reshape(S, P, nch, width).transpose(1, 2, 0, 3)
    return np.ascontiguousarray(y.reshape(P, nch * S * width))


def _bf16(x):
    import ml_dtypes
    return np.ascontiguousarray(x).astype(ml_dtypes.bfloat16)


def prepare_inputs(query, key, value, Wq_w, Wq_b, Wk_w, Wk_b, Wv_w, Wv_b,
                   Wo_w, Wo_b):
    query = np.asarray(query, dtype=np.float32)
    key = np.asarray(key, dtype=np.float32)
    value = np.asarray(value, dtype=np.float32)
    w = {
        "wq": _bf16(_chunked(np.ascontiguousarray(np.asarray(Wq_w, np.float32).T))),
        "wk": _bf16(_chunked(np.ascontiguousarray(np.asarray(Wk_w, np.float32).T))),
        "wv": _bf16(_chunked(np.ascontiguousarray(np.asarray(Wv_w, np.float32).T))),
        "wo": _bf16(_chunked(np.ascontiguousarray(np.asarray(Wo_w, np.float32).T))),
    }
    biases = {"bq": np.asarray(Wq_b, np.float32), "bk": np.asarray(Wk_b, np.float32),
              "bv": np.asarray(Wv_b, np.float32), "bo": np.asarray(Wo_b, np.float32)}
    has_bias = {nm: bool(np.any(b)) for nm, b in biases.items()}

    in_maps = []
    for c in range(N_CORES):
        b = c // 4
        qh = (c % 4) // 2
        kh = c % 2
        qs = slice(qh * LQ, (qh + 1) * LQ)
        ks = slice(kh * LK, (kh + 1) * LK)
        m = {
            "qT": _bf16(_chunked(np.ascontiguousarray(query[b, qs, :].T))),
            "kT": _bf16(_part_major(np.ascontiguousarray(key[b, ks, :].T))),
            "vT": _bf16(_part_major(np.ascontiguousarray(value[b, ks, :].T))),
            **w,
        }
        for nm, hb in has_bias.items():
            if hb:
                m[nm] = biases[nm].reshape(1, D)
        in_maps.append(m)
    return in_maps, has_bias


def gather_output(results):
    out = np.zeros((B, L, D), dtype=np.float32)
    for c in range(N_CORES):
        b = c // 4
        qh = (c % 4) // 2
        oT = results[c]["outT"].reshape(P, S, LQ).transpose(1, 0, 2).reshape(D, LQ)
        out[b, qh * LQ:(qh + 1) * LQ, :] += oT.T
    return out


def kernel(**inputs) -> np.ndarray:
    in_maps, has_bias = prepare_inputs(**inputs)
    nc = _get_program(has_bias)
    res = run_bass_kernel_spmd(nc, in_maps, list(range(N_CORES)))
    return gather_output(res.results)


# revision 19
# speedup vs baseline: 1.5660x; 1.0136x over previous
"""CategoryAttention (softmax over heads axis) on 8 Trainium2 cores.

Sharding: B*L = 4096 query rows split 8 ways (512 rows/core). Core c
handles batch b=c//4, query rows [(c%4)*512, (c%4+1)*512). Softmax is
over the 16 heads (local per (q,k) position) -> no cross-core comm.
Each core recomputes K/V projections for its batch (4x redundant).

Layout/schedule:
- All projections and attention matmuls in bf16 (FWL weight loads).
- Projections are FUSED into the attention sweep: K-chunk kn+1 and
  V-chunk kn are emitted between attention k-tiles so PE back-fills
  the ACT(exp)/DVE(normalize) pacing gaps and the HAM clock-gate
  rarely re-throttles.
- Energy matmuls row-pack two heads (partitions 0-63/64-127 ->
  concurrent PE row-groups); AV matmuls col-pack (psum halves).
- Proj psum drains use 1-bank tiles so drain overlaps accumulation.
- The reciprocal->bf16 cast runs on DVE (0.5us) instead of GPSIMD
  (1.9us): it sits on the per-k-tile critical chain.
- PE warm-keeper matmuls at kernel start (during input DMAs) and
  before the output projection defeat the HAM cold-clock penalty.
"""

import numpy as np
from contextlib import ExitStack

import concourse.bass as bass
import concourse.tile as tile
from concourse import bacc, mybir
from concourse.bass_utils import run_bass_kernel_spmd

F32 = mybir.dt.float32
BF16 = mybir.dt.bfloat16

N_CORES = 8
P = 128
D = 1024          # d_model
S = D // P        # 8 subtiles of the contraction dim
H = 16            # heads
HD = 64           # head dim
B = 2
L = 2048
LQ = L * B // N_CORES   # 512 query rows per core
LK = L                  # key rows per core (full batch slice)
KTS = 128               # k tile
NKT = LK // KTS         # 16
SCALE = 1.0 / np.sqrt(HD)

import os
BENCH_LOOP = int(os.environ.get("BENCH_LOOP", "1"))


def _build(has_bias):
    nc = bacc.Bacc("TRN2", target_bir_lowering=False, debug=False, num_devices=1)

    def din(name, shape, dt):
        return nc.dram_tensor(name, shape, dt, kind="ExternalInput").ap()

    qT_d = din("qT", (P, S * LQ), BF16)
    kT_d = din("kT", (P, 4 * S * 512), BF16)
    vT_d = din("vT", (P, 4 * S * 512), BF16)
    wq_d = din("wq", (P, 2 * S * 512), BF16)
    wk_d = din("wk", (P, 2 * S * 512), BF16)
    wv_d = din("wv", (P, 2 * S * 512), BF16)
    wo_d = din("wo", (P, 2 * S * 512), BF16)
    bias_d = {}
    for nm in ("bq", "bk", "bv", "bo"):
        if has_bias[nm]:
            bias_d[nm] = din(nm, (1, D), F32)
    outT_d = nc.dram_tensor("outT", (P, S * LQ), F32, kind="ExternalOutput").ap()

    qT_ap = qT_d.rearrange("p (s q) -> p s q", s=S)
    kT_ap = kT_d.rearrange("p (c s k) -> p c s k", c=4, s=S)
    vT_ap = vT_d.rearrange("p (c s k) -> p c s k", c=4, s=S)
    wq_ap = wq_d.rearrange("p (h s o) -> p h s o", h=2, s=S)
    wk_ap = wk_d.rearrange("p (h s o) -> p h s o", h=2, s=S)
    wv_ap = wv_d.rearrange("p (h s o) -> p h s o", h=2, s=S)
    wo_ap = wo_d.rearrange("p (h s o) -> p h s o", h=2, s=S)
    outT_ap = outT_d.rearrange("p (j q) -> p j q", j=S)

    with tile.TileContext(nc) as tc, ExitStack() as ctx:
        if BENCH_LOOP > 1:
            ctx.enter_context(tc.For_i(0, BENCH_LOOP, 1))

        # ---- persistent data tiles ----
        qt_pool = ctx.enter_context(tc.tile_pool(name="QT", bufs=1))
        kt_pool = ctx.enter_context(tc.tile_pool(name="KT", bufs=1))
        v_pool = ctx.enter_context(tc.tile_pool(name="V", bufs=1))
        QT_sb = qt_pool.tile([P, S, LQ], BF16)
        KT_sb = kt_pool.tile([P, S, LK], BF16)
        V_sb = v_pool.tile([P, NKT, D], BF16)

        any_bias = any(has_bias.values())
        bias_t = {}
        ones_t = None
        if any_bias:
            cpool = ctx.enter_context(tc.tile_pool(name="const", bufs=1))
            ones_t = cpool.tile([1, 512], F32, tag="ones")
            nc.vector.memset(ones_t[:], 1.0)
            for nm, d_ap in bias_d.items():
                t = cpool.tile([1, D], F32, tag=f"bias_{nm}")
                nc.sync.dma_start(t[:], d_ap)
                bias_t[nm] = t

        def bias_mm(ps_t, bias_name, o0, n_sz, o_on_partitions):
            if o_on_partitions:
                nc.tensor.matmul(ps_t, lhsT=bias_t[bias_name][0:1, o0:o0 + P],
                                 rhs=ones_t[0:1, :n_sz], start=False, stop=True)
            else:
                nc.tensor.matmul(ps_t, lhsT=ones_t[0:1, 0:P],
                                 rhs=bias_t[bias_name][0:1, o0:o0 + n_sz],
                                 start=False, stop=True)

        # psum pools (8 banks total: 2 proj + 4 energy + 2 av)
        ppsum = ctx.enter_context(tc.tile_pool(name="ppsum", bufs=2, space="PSUM"))
        e_psum = ctx.enter_context(tc.tile_pool(name="epsum", bufs=2, space="PSUM"))
        av_psum = ctx.enter_context(tc.tile_pool(name="avpsum", bufs=1, space="PSUM"))

        # PE warm-up: the HAM clock-gate boots at reduced rate; ~5us of dead
        # matmuls during the initial input DMAs un-throttle it so the first
        # projection matmuls run at full clock.
        warm_pool = ctx.enter_context(tc.tile_pool(name="warm", bufs=1))
        wrm = warm_pool.tile([P, 512], BF16, tag="wrm")
        nc.vector.memset(wrm[:], 0.0)
        for wk_i in range(12):
            wps = e_psum.tile([P, 2, LQ], F32, tag="e")
            for hh in range(2):
                nc.tensor.matmul(
                    wps[:, hh, :],
                    lhsT=wrm[:, 0:P],
                    rhs=wrm[:, :],
                    start=True,
                    stop=True,
                )

        # ---------------- Q projection (scoped: SBUF reused later) ----
        with tc.tile_pool(name="qstream", bufs=1) as qspool, \
             tc.tile_pool(name="qwpool", bufs=2) as qwpool:
            qin = qspool.tile([P, S, LQ], BF16, tag="qin")
            nc.sync.dma_start(qin[:], qT_ap)
            wq_h = []
            for wh in range(2):
                t = qwpool.tile([P, S, 512], BF16, tag="wq")
                nc.sync.dma_start(t[:], wq_ap[:, wh])
                wq_h.append(t)
            for j in range(S):
                ps = ppsum.tile([P, 1, 512], F32, tag="pp")
                w_t = wq_h[j // 4]
                jl = j % 4
                for s in range(S):
                    nc.tensor.matmul(
                        ps[:, 0, :LQ],
                        lhsT=w_t[:, s, jl * P:(jl + 1) * P],
                        rhs=qin[:, s, :],
                        start=(s == 0),
                        stop=(s == S - 1 and not has_bias["bq"]),
                    )
                if has_bias["bq"]:
                    bias_mm(ps[:, 0, :LQ], "bq", j * P, LQ, True)
                nc.scalar.copy(QT_sb[:, j, :], ps[:, 0, :LQ])

        # ---- attention-era pools (allocated after Q scope frees) ----
        wk_pool = ctx.enter_context(tc.tile_pool(name="wk", bufs=2))
        kin_pool = ctx.enter_context(tc.tile_pool(name="kin", bufs=1))
        wv_pool = ctx.enter_context(tc.tile_pool(name="wv", bufs=2))
        vin_pool = ctx.enter_context(tc.tile_pool(name="vin", bufs=1))
        wo_pool = ctx.enter_context(tc.tile_pool(name="wo", bufs=1))
        attn_pool = ctx.enter_context(tc.tile_pool(name="attn", bufs=3))
        tree_pool = ctx.enter_context(tc.tile_pool(name="tree", bufs=1))
        den_pool = ctx.enter_context(tc.tile_pool(name="den", bufs=1))
        r_pool = ctx.enter_context(tc.tile_pool(name="r", bufs=2))
        rb_pool = ctx.enter_context(tc.tile_pool(name="rb", bufs=2))
        ctx_pool = ctx.enter_context(tc.tile_pool(name="ctx", bufs=1))
        osb_pool = ctx.enter_context(tc.tile_pool(name="osb", bufs=2))

        ctx_sb = ctx_pool.tile([P, S, LQ], BF16)

        wk_h = []
        for wh in range(2):
            t = wk_pool.tile([P, S, 512], BF16, tag="wk")
            nc.sync.dma_start(t[:], wk_ap[:, wh])
            wk_h.append(t)
        wv_h = []
        for wh in range(2):
            t = wv_pool.tile([P, S, 512], BF16, tag="wv")
            nc.sync.dma_start(t[:], wv_ap[:, wh])
            wv_h.append(t)

        kin_cur = [None]
        vin_cur = [None]

        def dma_kin(kn):
            t = kin_pool.tile([P, S, 512], BF16, tag="kin")
            nc.sync.dma_start(t[:], kT_ap[:, kn])
            kin_cur[0] = t

        def dma_vin(kn):
            t = vin_pool.tile([P, S, 512], BF16, tag="vin")
            nc.sync.dma_start(t[:], vT_ap[:, kn])
            vin_cur[0] = t

        def k_chunk_quarter(kn, jq):
            """Project K columns [kn*512,(kn+1)*512) for o-tiles 2jq,2jq+1."""
            kin = kin_cur[0]
            for j in (2 * jq, 2 * jq + 1):
                ps = ppsum.tile([P, 1, 512], F32, tag="pp")
                w_t = wk_h[j // 4]
                jl = j % 4
                for s in range(S):
                    nc.tensor.matmul(
                        ps[:, 0, :],
                        lhsT=w_t[:, s, jl * P:(jl + 1) * P],
                        rhs=kin[:, s, :],
                        start=(s == 0),
                        stop=(s == S - 1 and not has_bias["bk"]),
                    )
                if has_bias["bk"]:
                    bias_mm(ps[:, 0, :], "bk", j * P, 512, True)
                nc.scalar.copy(KT_sb[:, j, kn * 512:(kn + 1) * 512], ps[:, 0, :])

        def v_chunk_quarter(kn, kt4):
            """Project V rows for k-tile kn*4+kt4 (128 rows, all 1024 cols)."""
            vin = vin_cur[0]
            kt = kn * 4 + kt4
            for t in range(2):
                ps = ppsum.tile([P, 1, 512], F32, tag="pp")
                for s in range(S):
                    nc.tensor.matmul(
                        ps[:, 0, :],
                        lhsT=vin[:, s, kt4 * P:(kt4 + 1) * P],
                        rhs=wv_h[t][:, s, :],
                        start=(s == 0),
                        stop=(s == S - 1 and not has_bias["bv"]),
                    )
                if has_bias["bv"]:
                    bias_mm(ps[:, 0, :], "bv", t * 512, 512, False)
                nc.scalar.copy(V_sb[:, kt, t * 512:(t + 1) * 512], ps[:, 0, :])

        # ---------------- attention ----------------
        def softmax_kt(kt):
            """Energy (16 heads, row-packed pairs) -> exp -> normalized attn."""
            attn_t = attn_pool.tile([P, H, LQ], BF16, tag="attn")
            for g in range(8):
                eps = e_psum.tile([P, 2, LQ], F32, tag="e")
                for hh in range(2):
                    p0 = HD * hh
                    nc.tensor.matmul(
                        eps[:, hh, :],
                        lhsT=KT_sb[p0:p0 + HD, g, kt * KTS:(kt + 1) * KTS],
                        rhs=QT_sb[p0:p0 + HD, g, :],
                        start=True,
                        stop=True,
                    )
                nc.scalar.activation(attn_t[:, g * 2:(g + 1) * 2, :], eps[:],
                                     mybir.ActivationFunctionType.Exp,
                                     scale=float(SCALE))
            # den = sum over heads (bf16 tree at DVE 2x; final add f32)
            t1 = tree_pool.tile([P, 4, LQ], BF16)
            with nc.allow_low_precision(reason="bf16 head-sum tree"):
                nc.vector.tensor_add(t1[:], attn_t[:, 0:4, :], attn_t[:, 4:8, :])
                nc.vector.tensor_add(t1[:], t1[:], attn_t[:, 8:12, :])
                nc.vector.tensor_add(t1[:], t1[:], attn_t[:, 12:16, :])
                nc.vector.tensor_add(t1[:, 0:2, :], t1[:, 0:2, :], t1[:, 2:4, :])
            den = den_pool.tile([P, LQ], F32)
            nc.vector.tensor_add(den[:], t1[:, 0, :], t1[:, 1, :])
            r32 = r_pool.tile([P, LQ], F32, tag="r")
            nc.vector.reciprocal_approx_fast(r32[:], den[:])
            rb = rb_pool.tile([P, LQ], BF16, tag="rb")
            with nc.allow_low_precision(reason="bf16 reciprocal"):
                nc.vector.tensor_copy(rb[:], r32[:])
            nc.vector.tensor_mul(
                attn_t[:], attn_t[:],
                rb[:, None, :].to_broadcast((P, H, LQ)))
            return attn_t

        def av_group(u, c0, attn_list, first):
            """One avp tile: heads 4u..4u+3, full q, over 2 k-tiles."""
            avp = av_psum.tile([P, 2, LQ], F32, tag="av")
            for ci in range(2):
                kt = c0 + ci
                for hh in range(4):
                    h = 4 * u + hh
                    i, p0 = hh // 2, HD * (hh % 2)
                    nc.tensor.matmul(
                        avp[p0:p0 + HD, i, :],
                        lhsT=V_sb[:, kt, h * HD:(h + 1) * HD],
                        rhs=attn_list[ci][:, h, :],
                        start=(ci == 0),
                        stop=(ci == 1),
                    )
            with nc.allow_low_precision(reason="bf16 ctx accumulate"):
                if first:
                    nc.vector.tensor_copy(ctx_sb[:, 2 * u:2 * u + 2, :],
                                          avp[:, :, :])
                else:
                    nc.vector.tensor_add(ctx_sb[:, 2 * u:2 * u + 2, :],
                                         ctx_sb[:, 2 * u:2 * u + 2, :],
                                         avp[:, :, :])

        wo_tiles = []

        def dma_wo0():
            t = wo_pool.tile([P, S, 512], BF16, tag="wo")
            nc.sync.dma_start(t[:], wo_ap[:, 0])
            wo_tiles.append(t)

        # filler schedule: per-kt projection quarters + input DMAs
        def proj_filler(kt):
            if kt == 0:
                dma_kin(1)
                v_chunk_quarter(0, 0); v_chunk_quarter(0, 1)
            elif kt == 1:
                v_chunk_quarter(0, 2); v_chunk_quarter(0, 3)
                dma_vin(1)
            elif kt in (2, 3, 6, 7, 10, 11):
                kn = kt // 4 + 1
                jq0 = 0 if kt % 4 == 2 else 2
                k_chunk_quarter(kn, jq0); k_chunk_quarter(kn, jq0 + 1)
                if kt in (3, 7):
                    dma_kin(kn + 1)
            elif kt in (4, 5, 8, 9, 12, 13):
                kn = kt // 4
                kt40 = 0 if kt % 4 == 0 else 2
                v_chunk_quarter(kn, kt40); v_chunk_quarter(kn, kt40 + 1)
                if kt in (5, 9):
                    dma_vin(kn + 1)
                if kt == 12:
                    dma_wo0()
            # kt 14, 15: no proj work left

        # prologue: K chunk 0 (all 8 o-tiles)
        dma_kin(0)
        dma_vin(0)
        k_chunk_quarter(0, 0); k_chunk_quarter(0, 1)
        k_chunk_quarter(0, 2); k_chunk_quarter(0, 3)

        prev = None  # (c0, [attn_kt0, attn_kt1])
        for p in range(8):
            c0 = 2 * p
            cur = []
            for ci in range(2):
                kt = c0 + ci
                cur.append(softmax_kt(kt))
                if prev is not None:
                    for u in (2 * ci, 2 * ci + 1):
                        av_group(u, prev[0], prev[1], prev[0] == 0)
                proj_filler(kt)
            prev = (c0, cur)
        # PE warm-keeper: ~4us of dead matmuls run while the DVE finishes
        # the last normalize, so the HAM clock-gate stays at full rate for
        # the final AV groups + output projection.
        for wk_i in range(10):
            wps = e_psum.tile([P, 2, LQ], F32, tag="e")
            for hh in range(2):
                p0 = HD * hh
                nc.tensor.matmul(
                    wps[:, hh, :],
                    lhsT=KT_sb[p0:p0 + HD, wk_i % S, 0:KTS],
                    rhs=QT_sb[p0:p0 + HD, wk_i % S, :],
                    start=True,
                    stop=True,
                )
        for u in range(4):
            av_group(u, prev[0], prev[1], False)

        # ---------------- output projection ----------------
        for j4 in range(2):
            if j4 == 0 and wo_tiles:
                woh = wo_tiles[0]
            else:
                woh = wo_pool.tile([P, S, 512], BF16, tag="wo")
                nc.sync.dma_start(woh[:], wo_ap[:, j4])
            for j2 in range(2):
                po = e_psum.tile([P, 2, LQ], F32, tag="e")
                for jj in range(2):
                    j = j4 * 4 + j2 * 2 + jj
                    jl = j2 * 2 + jj
                    for s in range(S):
                        nc.tensor.matmul(
                            po[:, jj, :],
                            lhsT=woh[:, s, jl * P:(jl + 1) * P],
                            rhs=ctx_sb[:, s, :],
                            start=(s == 0),
                            stop=(s == S - 1 and not has_bias["bo"]),
                        )
                    if has_bias["bo"]:
                        bias_mm(po[:, jj, :], "bo", j * P, LQ, True)
                osb = osb_pool.tile([P, 2, LQ], F32, tag="osb")
                nc.scalar.copy(osb[:], po[:])
                j0 = j4 * 4 + j2 * 2
                nc.sync.dma_start(outT_ap[:, j0:j0 + 2, :], osb[:])

    nc.compile()
    return nc


_cache = {}


def _get_program(has_bias):
    key = (BENCH_LOOP, tuple(sorted(has_bias.items())))
    if key not in _cache:
        _cache[key] = _build(has_bias)
    return _cache[key]


def _part_major(x):
    n = x.shape[1]
    return np.ascontiguousarray(
        x.reshape(S, P, n).transpose(1, 0, 2).reshape(P, S * n))


def _chunked(x, width=512):
    """[D, N] -> [P, N//width, S, width] per-chunk contiguous layout."""
    n = x.shape[1]
    nch = n // width
    y = x.reshape(S, P, nch, width).transpose(1, 2, 0, 3)
    return np.ascontiguousarray(y.reshape(P, nch * S * width))


def _bf16(x):
    import ml_dtypes
    return np.ascontiguousarray(x).astype(ml_dtypes.bfloat16)


def prepare_inputs(query, key, value, Wq_w, Wq_b, Wk_w, Wk_b, Wv_w, Wv_b,
                   Wo_w, Wo_b):
    query = np.asarray(query, dtype=np.float32)
    key = np.asarray(key, dtype=np.float32)
    value = np.asarray(value, dtype=np.float32)
    w = {
        "wq": _bf16(_chunked(np.ascontiguousarray(np.asarray(Wq_w, np.float32).T))),
        "wk": _bf16(_chunked(np.ascontiguousarray(np.asarray(Wk_w, np.float32).T))),
        "wv": _bf16(_chunked(np.ascontiguousarray(np.asarray(Wv_w, np.float32).T))),
        "wo": _bf16(_chunked(np.ascontiguousarray(np.asarray(Wo_w, np.float32).T))),
    }
    biases = {"bq": np.asarray(Wq_b, np.float32), "bk": np.asarray(Wk_b, np.float32),
              "bv": np.asarray(Wv_b, np.float32), "bo": np.asarray(Wo_b, np.float32)}
    has_bias = {nm: bool(np.any(b)) for nm, b in biases.items()}

    kT = [_bf16(_chunked(np.ascontiguousarray(key[b].T))) for b in range(B)]
    vT = [_bf16(_chunked(np.ascontiguousarray(value[b].T))) for b in range(B)]

    in_maps = []
    for c in range(N_CORES):
        b, qc = c // (N_CORES // B), c % (N_CORES // B)
        qslice = query[b, qc * LQ:(qc + 1) * LQ, :]
        m = {
            "qT": _bf16(_part_major(np.ascontiguousarray(qslice.T))),
            "kT": kT[b],
            "vT": vT[b],
            **w,
        }
        for nm, hb in has_bias.items():
            if hb:
                m[nm] = biases[nm].reshape(1, D)
        in_maps.append(m)
    return in_maps, has_bias


def gather_output(results):
    out = np.empty((B, L, D), dtype=np.float32)
    for c in range(N_CORES):
        b, qc = c // (N_CORES // B), c % (N_CORES // B)
        oT = results[c]["outT"].reshape(P, S, LQ).transpose(1, 0, 2).reshape(D, LQ)
        out[b, qc * LQ:(qc + 1) * LQ, :] = oT.T
    return out


def kernel(**inputs) -> np.ndarray:
    in_maps, has_bias = prepare_inputs(**inputs)
    nc = _get_program(has_bias)
    res = run_bass_kernel_spmd(nc, in_maps, list(range(N_CORES)))
    return gather_output(res.results)
